# revision 26
# baseline (speedup 1.0000x reference)
"""MultiHeadAttention forward on 8 Trainium2 NeuronCores (Bass/Tile).

Problem (hardcoded): B=2, S=2048, D=1024, H=16, HD=64.
  qkv = x @ w_qkv.T + b_qkv ; per-head attention with softmax(q k^T/8 + mask);
  out = values @ w_out.T + b_out.

Sharding: tensor-parallel over heads -- core c owns heads {2c, 2c+1}
(value dims 128c..128c+127).  Each core computes its 2 heads end-to-end and
a partial output projection; the host sums the 8 partials (bf16) and adds
the bias constant (b_out + b_v @ w_out.T, exact because softmax rows sum
to 1, and q.bk-type score shifts are softmax-invariant).

Device layout notes:
 - everything bf16 on the PE (same PE rate as f32r, half the DMA/SBUF).
 - scores are computed TRANSPOSED (S^T[k,tq] = K^T.T @ Q^T per head); the
   two heads' score matmuls hit disjoint PE row groups (partitions 0-63 /
   64-127) and run concurrently.
 - the exp stream is the serial bottleneck (128 x ~1.15us ACTIVATE =
   147us; ACT cost = (N+352)/1.2GHz, dtype-independent, and only ScalarE
   has activation LUTs), so everything is built to never stall it:
   * scores are emitted DEPTH=3 chunks ahead of AV in a flat
     (block, chunk) loop crossing block boundaries, and pt (probs) has
     an 8-deep pool: heavy Phase-A filler items make the in-order PE
     queue lag ~2-6us, and a 1-deep score pipeline lags with it.
   * the whole block epilogue (normalize + out-projection + out DMA) is
     queued as FILLER pulled during the NEXT block; emitted inline it
     head-blocks the next block's scores in the PE queue.
   * av is staged out of PSUM with cheap copies FIRST: the 2-buffer av
     pool is reused every block, and holding a buffer through the
     normalize chain stalls the next block's AV accumulation.
 - vext carries 32 ones columns so AV rows 64..95 hold the softmax
   denominator l; 32x32 DVE block transposes make l partition-parallel
   (DVE reciprocal costs ~6.5 cycles per FREE element -- 3.3us for
   [1,512] -- so the transposes are load-bearing), then a K=1 bf16 PE
   matmul broadcasts 1/l (bf16 here adds ~0.2% relative: fine vs the
   2e-2 gate; f32r matmuls measured ~700ns vs ~320ns bf16) and one DVE
   multiply (av_sb SBUF x bcp PSUM -- DVE allows ONE PSUM operand)
   writes valsT.
 - valsT is one [128, T] tile (head h on partitions 64h..64h+63) so the
   output projection is a single K=128 matmul per out-tile.
 - GpSimd is useless here: it has NO PSUM port (walrus rejects it) and
   is ~3x slower than DVE on small SBUF ops (1861ns for a [1,512] cast).
 - host pre-arranges x^T/wqk/wv/out into partition-major blocks so every
   DMA is a contiguous >=4KB run per partition (the naive (c p) j gather
   ran at ~73GB/s); even so a queue moves only ~137GB/s (descriptor
   setup bound), so the x stream rides the Activation HWDGE queue in
   parallel with the weight stream on the SP queue at startup.
 - ~7 dummy matmuls on a memset tile run first to flip the PE HAM clock
   gate (1.2 -> 2.4 GHz takes ~3.4us of sustained busy; >3.4us idle
   drops it back) while the first DMAs land.
 - the LAST block's epilogue has nothing to overlap: half its casts run
   on the idle ScalarE and its out DMA is split in two.
 - K=64 row-paired AV was analyzed useless (matmul time = N cycles
   regardless of K) and alternating PE tile configs inside one PSUM
   accumulation group crashes the NEFF execution on HW -- not used.
 - Custom-DVE ops (reciprocal_approx_fast) and GpSimd ucode ops
   (partition_broadcast) misbehave on this runtime -- plain ops only.
"""
import sys
if "/opt/trn_rl_repo" not in sys.path:
    sys.path.insert(0, "/opt/trn_rl_repo")
import numpy as np
from collections import deque

B, S, D, H = 2, 2048, 1024, 16
HD = D // H           # 64
NCORES = 8
T = B * S             # 4096 tokens
NB = S // 512         # 4 tq blocks per batch
NCH = S // 128        # 16 kpos chunks per batch
NBLK = B * NB         # 8 attention blocks total

_CACHE = {}


def build_nc(use_mask: bool, reps: int = 1):
    """Build + compile the per-core Bass program (SPMD-identical)."""
    import concourse.bacc as bacc
    import concourse.tile as tile
    from concourse import mybir

    f32 = mybir.dt.float32
    f32r = mybir.dt.float32r
    bf16 = mybir.dt.bfloat16
    EXP = mybir.ActivationFunctionType.Exp
    MULT = mybir.AluOpType.mult

    nc = bacc.Bacc("TRN2", target_bir_lowering=False, debug=False,
                   num_devices=NCORES)

    xTb = nc.dram_tensor("xTb", (8, 128, 8, 512), bf16, kind="ExternalInput")
    wqkT = nc.dram_tensor("wqkT", (128, 8, 256), bf16, kind="ExternalInput")
    bqk = nc.dram_tensor("bqk", (128, 2), f32, kind="ExternalInput")
    wvT = nc.dram_tensor("wvT", (128, 8, 128), bf16, kind="ExternalInput")
    woT = nc.dram_tensor("woT", (128, D), bf16, kind="ExternalInput")
    identb = nc.dram_tensor("identb", (128, 128), bf16, kind="ExternalInput")
    if use_mask:
        maskT = nc.dram_tensor("maskT", (B, S, S), f32r, kind="ExternalInput")
        ident = nc.dram_tensor("ident", (128, 128), f32r, kind="ExternalInput")
    out = nc.dram_tensor("out", (NBLK, 128, 4, 2, 512), bf16,
                         kind="ExternalOutput")

    with tile.TileContext(nc) as tc:
        with tc.tile_pool(name="sbp", bufs=1) as sbp, \
             tc.tile_pool(name="xtbp", bufs=3) as xtbp, \
             tc.tile_pool(name="ptp", bufs=8) as ptp, \
             tc.tile_pool(name="lrp", bufs=2) as lrp, \
             tc.tile_pool(name="otp", bufs=2) as otp, \
             tc.tile_pool(name="mkp", bufs=4) as mkp, \
             tc.tile_pool(name="mmp", bufs=2, space="PSUM") as mmp, \
             tc.tile_pool(name="scp", bufs=2, space="PSUM") as scp, \
             tc.tile_pool(name="avp", bufs=2, space="PSUM") as avp:

            # --- persistent SBUF tensors ---
            qkt = sbp.tile([128, 2, T], bf16, name="qkt")    # [feat,{q,k},tok]
            vext = sbp.tile([128, B, 2, NCH, HD + 32], bf16, name="vext")
            vT_sb = sbp.tile([128, T], bf16, name="vT_sb")   # [vfeat, tok]
            valsT = sbp.tile([128, T], bf16, name="valsT")   # [64h+d, tok]
            wqk_sb = sbp.tile([128, 8, 256], bf16, name="wqk_sb")
            wv_sb = sbp.tile([128, 8, 128], bf16, name="wv_sb")
            wo_sb = sbp.tile([128, D], bf16, name="wo_sb")
            bqk_sb = sbp.tile([128, 2], f32, name="bqk_sb")
            idb_sb = sbp.tile([128, 128], bf16, name="idb_sb")
            ones_sb = sbp.tile([65, 64], bf16, name="ones_sb")
            warm_sb = sbp.tile([128, 512], bf16, name="warm_sb")
            nc.vector.memset(ones_sb, 1.0)
            nc.vector.memset(warm_sb, 0.0)
            nc.vector.memset(vext[:, :, :, :, HD:HD + 32], 1.0)

            # --- PE HAM warm-up: ~7 dummy matmuls (~3us at the cold
            # 1.2 GHz clock) start flipping the clock gate to 2.4 GHz
            # while the input DMAs land; real qk work abuts and finishes
            # the 3.4us sustained-busy window.  One accumulation group.
            warm_ps = mmp.tile([128, 512], f32, tag="mm", name="warm_ps")
            for i in range(9):
                nc.tensor.matmul(warm_ps, warm_sb[:, 0:128], warm_sb,
                                 start=(i == 0), stop=(i == 8))
            # dummy reader (the BIR verifier rejects never-read PSUM)
            nc.vector.tensor_copy(warm_sb[0:1, 0:1], warm_ps[0:1, 0:1])

            # --- startup DMAs, ordered by first use: the first qk matmul
            # needs x(tb0) + wqk + bqk; everything else follows.
            # All DMAs on the SP queue: the Activation HWDGE queue was
            # measured ~3x slower (256KB in 7.4us), so parallel-queue
            # splitting loses.
            xtb0 = xtbp.tile([128, 8, 512], bf16, tag="xtb", name="xtb_first")
            for lo in (0, 4):
                nc.sync.dma_start(xtb0[:, lo:lo + 4, :],
                                  xTb[0, :, lo:lo + 4, :])
            nc.sync.dma_start(wqk_sb, wqkT[:, :, :])
            nc.sync.dma_start(bqk_sb, bqk[:, :])
            if use_mask:
                id_sb = sbp.tile([128, 128], f32r, name="id_sb")
                nc.sync.dma_start(id_sb, ident[:, :])
            nc.sync.dma_start(wv_sb, wvT[:, :, :])
            nc.sync.dma_start(idb_sb, identb[:, :])
            nc.sync.dma_start(wo_sb, woT[:, :])

            def phase_a_block(rep, b, tb, first=False):
                """Emit the list of closures for one 512-token projection
                block (tb in 0..7 global).  Returned items are emitted
                lazily as filler (or inline for tb0)."""
                items = []
                if first:
                    xtb_t = xtb0
                else:
                    xtb_t = xtbp.tile([128, 8, 512], bf16, tag="xtb",
                                      name=f"xtb_{rep}_{tb}")

                    def load(tb=tb, xtb_t=xtb_t):
                        nc.sync.dma_start(xtb_t, xTb[tb, :, :, :])
                    items.append(load)

                for m in range(2):          # q then k projections
                    def qk(m=m):
                        acc = mmp.tile([128, 512], f32, tag="mm",
                                       name=f"qk_{rep}_{tb}_{m}")
                        for c in range(8):
                            nc.tensor.matmul(
                                acc, wqk_sb[:, c, 128 * m:128 * m + 128],
                                xtb_t[:, c, :], start=(c == 0), stop=(c == 7))
                        nc.vector.tensor_scalar_add(
                            qkt[:, m, 512 * tb:512 * tb + 512], acc,
                            bqk_sb[:, m:m + 1])
                    items.append(qk)

                def vpass():
                    # v^T [vfeat, tok] with wv stationary (long streams,
                    # weight loads hidden), then cast to bf16 SBUF.
                    vacc = mmp.tile([128, 512], f32, tag="mm",
                                    name=f"vacc_{rep}_{tb}")
                    for c in range(8):
                        nc.tensor.matmul(
                            vacc, wv_sb[:, c, :], xtb_t[:, c, :],
                            start=(c == 0), stop=(c == 7))
                    nc.vector.tensor_copy(
                        vT_sb[:, 512 * tb:512 * tb + 512], vacc)
                items.append(vpass)

                for u in range(4):          # transpose to [kpos, feat] tiles
                    def vtrans(u=u):
                        t0g = 512 * tb + 128 * u
                        cc = (t0g % S) // 128
                        vtp = mmp.tile([128, 128], f32, tag="mm",
                                       name=f"vtp_{rep}_{tb}_{u}")
                        nc.tensor.matmul(vtp, vT_sb[:, t0g:t0g + 128], idb_sb,
                                         start=True, stop=True)
                        nc.vector.tensor_copy(
                            vext[:, b, :, cc, 0:HD],
                            vtp[:, :].rearrange("p (h d) -> p h d", h=2))
                    items.append(vtrans)
                return items

            def pull(filler, n):
                for _ in range(n):
                    if not filler:
                        return
                    filler.popleft()()

            def make_ctx(rep, i):
                b, tqb = divmod(i, NB)
                tq0 = S * b + 512 * tqb
                q_aps = [qkt[64 * h:64 * h + 64, 0, tq0:tq0 + 512]
                         for h in range(2)]
                avs = [avp.tile([96, 512], f32, tag="av",
                                name=f"av_{rep}_{b}_{tqb}_{h}")
                       for h in range(2)]
                return dict(b=b, tqb=tqb, tq0=tq0, q_aps=q_aps, avs=avs)

            def emit_scores(rep, ctx, c):
                b, tqb = ctx["b"], ctx["tqb"]
                sc = scp.tile([128, 1024], f32, tag="sc",
                              name=f"sc_{rep}_{b}_{tqb}_{c}")
                for h in range(2):
                    k_ap = qkt[64 * h:64 * h + 64, 1,
                               S * b + 128 * c:S * b + 128 * c + 128]
                    nc.tensor.matmul(
                        sc[:, 512 * h:512 * h + 512], k_ap, ctx["q_aps"][h],
                        start=True, stop=(not use_mask))
                if use_mask:
                    mt = mkp.tile([128, 512], f32r, tag="mk",
                                  name=f"mk_{rep}_{b}_{tqb}_{c}")
                    nc.sync.dma_start(
                        mt, maskT[b, 128 * c:128 * c + 128,
                                  512 * tqb:512 * tqb + 512])
                    for h in range(2):
                        nc.tensor.matmul(
                            sc[:, 512 * h:512 * h + 512], id_sb, mt,
                            start=False, stop=True)
                return sc

            def queue_block_epilogue(rep, ctx, filler, last=False):
                """Normalize + output projection + out DMA for a finished
                block, all as filler items pulled during the next block.
                For the LAST block (nothing left to overlap) half the
                PSUM->SBUF casts run on the now-idle ScalarE and the out
                DMA is split so transfers overlap the remaining casts."""
                b, tqb, tq0, avs = ctx["b"], ctx["tqb"], ctx["tq0"], ctx["avs"]
                blk = b * NB + tqb

                for h in range(2):
                    def norm(h=h):
                        # av rows 64..95 all hold l (32 ones columns in
                        # vext); 32x32 DVE block transposes make l
                        # partition-parallel because the DVE reciprocal
                        # costs ~6.5 cycles per FREE element (measured
                        # 3.3us for [1,512] -- the transposes are load-
                        # bearing), then a K=1 f32r PE matmul broadcasts
                        # 1/l across partitions for the DVE normalize.
                        # av is copied out of PSUM first so the (doubly-
                        # scarce) av buffer frees immediately: the NEXT
                        # block's AV accumulation reuses this buffer, and
                        # holding it head-blocks the in-order PE queue.
                        av = avs[h]
                        av_sb = lrp.tile([64, 512], f32, tag="avs",
                                         name=f"avs_{rep}_{b}_{h}_{tqb}")
                        nc.vector.tensor_copy(av_sb, av[0:64, :])
                        lt = lrp.tile([96, 512], f32, tag="lt",
                                      name=f"lt_{rep}_{b}_{h}_{tqb}")
                        nc.vector.transpose(lt[64:96, :], av[64:96, :])
                        lt3 = lt[64:96, :].rearrange(
                            "p (a b) -> p a b", b=32)[:, :, 0:1]
                        nc.vector.reciprocal(lt3, lt3)
                        rlrowf = lrp.tile([96, 512], f32, tag="rlrowf",
                                          name=f"rlrowf_{rep}_{b}_{h}_{tqb}")
                        nc.vector.transpose(rlrowf[64:96, :], lt[64:96, :])
                        # 1/l broadcast in bf16 (f32r matmuls measured
                        # ~700ns vs ~320ns bf16; 1/l at bf16 adds ~0.2%
                        # relative -- rel err stays well under 2e-2).  All
                        # on DVE: GpSimd is ~3x slower on these small ops
                        # (measured 1861ns for a [1,512] cast) and this
                        # chain gates the next block's outproj filler.
                        rlrow = lrp.tile([65, 512], bf16, tag="rlrow",
                                         name=f"rlrow_{rep}_{b}_{h}_{tqb}")
                        nc.vector.tensor_copy(rlrow[64:65, :],
                                              rlrowf[64:65, :])
                        bcp = mmp.tile([128, 512], f32, tag="mm",
                                       name=f"bcp_{rep}_{b}_{tqb}_{h}")
                        nc.tensor.matmul(
                            bcp[0:64, :], ones_sb[64:65, :],
                            rlrow[64:65, :], start=True, stop=True)
                        nc.vector.tensor_tensor(
                            valsT[64 * h:64 * h + 64, tq0:tq0 + 512],
                            av_sb, bcp[0:64, :], MULT)
                    filler.append(norm)

                ot = otp.tile([128, 4, 2, 512], bf16, tag="ot",
                              name=f"ot_{rep}_{b}_{tqb}")
                for u in range(4):
                    for nb2 in range(2):
                        def op_item(u=u, nb2=nb2):
                            t0 = tq0 + 128 * u
                            op = mmp.tile([128, 512], f32, tag="mm",
                                          name=f"op_{rep}_{b}_{tqb}_{nb2}_{u}")
                            nc.tensor.matmul(
                                op, valsT[:, t0:t0 + 128],
                                wo_sb[:, 512 * nb2:512 * nb2 + 512],
                                start=True, stop=True)
                            if last and nb2 == 1:
                                nc.scalar.copy(ot[:, u, nb2, :], op)
                            else:
                                nc.vector.tensor_copy(ot[:, u, nb2, :], op)
                        filler.append(op_item)
                    if last and u == 1:
                        def half_dma():
                            nc.sync.dma_start(out[blk, :, 0:2, :, :],
                                              ot[:, 0:2, :, :])
                        filler.append(half_dma)

                def out_dma():
                    if last:
                        nc.sync.dma_start(out[blk, :, 2:4, :, :],
                                          ot[:, 2:4, :, :])
                    else:
                        nc.sync.dma_start(out[blk, :, :, :, :], ot)
                filler.append(out_dma)

            for rep in range(reps):
                filler = deque()
                # Only tb0 of Phase A is emitted directly: attention(b0,
                # tqb0) needs just the first k/v chunks, so tb1-3 stream in
                # as filler during its chunk loop (chunk 4c needs tb c,
                # pulled 2 items/chunk -> arrives just in time).  Batch 1's
                # blocks become filler for the later b0 attention blocks.
                for item in phase_a_block(rep, 0, 0, first=True):
                    item()
                for tb in range(1, 4):
                    # fire tb1-3's x DMAs now (right behind the weights in
                    # the queue; the relayout makes each a ~3us contiguous
                    # transfer) -- chunk 4c of b0 attention needs tb c.
                    items = phase_a_block(rep, 0, tb)
                    items.pop(0)()
                    filler.extend(items)

                # Flat (block, chunk) sequence with scores emitted DEPTH=3
                # chunks ahead of AV (across block boundaries): the exp
                # stream only depends on scores, and when heavy Phase-A
                # filler makes the in-order PE queue lag, a 1-deep score
                # pipeline lags with it and stalls ScalarE.  Depth 3 plus
                # the 8-deep pt pool decouples exp from PE lag.
                DEPTH = 3
                seq = [(i, c) for i in range(NBLK) for c in range(NCH)]
                ctxs = {}

                def get_ctx(i):
                    if i not in ctxs:
                        ctxs[i] = make_ctx(rep, i)
                    return ctxs[i]

                sc_q = deque()
                for j in range(DEPTH):
                    sc_q.append(emit_scores(rep, get_ctx(seq[j][0]),
                                            seq[j][1]))
                for idx, (i, c) in enumerate(seq):
                    if idx + DEPTH < len(seq):
                        i2, c2 = seq[idx + DEPTH]
                        sc_q.append(emit_scores(rep, get_ctx(i2), c2))
                    pull(filler, 2)
                    ctx = get_ctx(i)
                    pt = ptp.tile([128, 1024], bf16, tag="pt",
                                  name=f"pt_{rep}_{i}_{c}")
                    nc.scalar.activation(pt, sc_q.popleft(), EXP)
                    for h in range(2):
                        nc.tensor.matmul(
                            ctx["avs"][h], vext[:, ctx["b"], h, c, :],
                            pt[:, 512 * h:512 * h + 512],
                            start=(c == 0), stop=(c == NCH - 1))
                    if c == NCH - 1:
                        queue_block_epilogue(rep, ctx, filler,
                                             last=(i == NBLK - 1))
                        if ctx["b"] == 0:
                            filler.extend(
                                phase_a_block(rep, 1, 4 + ctx["tqb"]))
                while filler:
                    filler.popleft()()
    nc.compile()
    return nc


def make_in_maps(mha_x, self_mask, w_qkv, b_qkv, w_out, b_out, use_mask):
    """Host-side sharding / layout prep. Returns (in_maps, host_bias)."""
    import ml_dtypes
    bf = np.dtype(ml_dtypes.bfloat16)
    x = np.asarray(mha_x, np.float32).reshape(T, D)
    # x^T pre-arranged [tb, p, c, j] so each 512-token block is one
    # contiguous-per-partition DMA (8KB rows; the naive (c p) j gather
    # ran at ~73 GB/s, ~8us for 512KB)
    xTb_np = np.ascontiguousarray(
        x.T.reshape(8, 128, 8, 512).transpose(2, 1, 0, 3).astype(bf))
    scale = 1.0 / np.sqrt(np.float32(HD))               # 1/8
    wqkv = np.asarray(w_qkv, np.float32)
    bqkv = np.asarray(b_qkv, np.float32)
    wout = np.asarray(w_out, np.float32)
    bout = np.asarray(b_out, np.float32)

    # reference packs w_qkv rows as [H, (q,k,v), HD]: head h's q rows are
    # wqkv[192h:192h+64], k rows +64, v rows +128.
    wq_rows = lambda h: wqkv[192 * h:192 * h + 64, :]
    wk_rows = lambda h: wqkv[192 * h + 64:192 * h + 128, :]
    wv_rows = lambda h: wqkv[192 * h + 128:192 * h + 192, :]
    bq_of = lambda h: bqkv[192 * h:192 * h + 64]
    bk_of = lambda h: bqkv[192 * h + 64:192 * h + 128]
    bv_of = lambda h: bqkv[192 * h + 128:192 * h + 192]

    in_maps = []
    for c in range(NCORES):
        h0, h1 = 2 * c, 2 * c + 1
        wq = np.concatenate([wq_rows(h0), wq_rows(h1)], 0) * scale
        wk = np.concatenate([wk_rows(h0), wk_rows(h1)], 0)
        wv = np.concatenate([wv_rows(h0), wv_rows(h1)], 0)
        m = {
            "xTb": xTb_np,
            "wqkT": np.ascontiguousarray(
                np.concatenate([wq, wk], 0).T.reshape(
                    8, 128, 256).transpose(1, 0, 2).astype(bf)),
            "bqk": np.ascontiguousarray(
                np.stack([np.concatenate([bq_of(h0), bq_of(h1)]) * scale,
                          np.concatenate([bk_of(h0), bk_of(h1)])], 1)),
            "wvT": np.ascontiguousarray(
                wv.T.reshape(8, 128, 128).transpose(1, 0, 2).astype(bf)),
            "woT": np.ascontiguousarray(
                wout[:, 128 * c:128 * c + 128].T.astype(bf)),
            "identb": np.eye(128, dtype=np.float32).astype(bf),
        }
        if use_mask:
            m["maskT"] = np.ascontiguousarray(
                np.asarray(self_mask, np.float32).transpose(0, 2, 1))
            m["ident"] = np.eye(128, dtype=np.float32)
        in_maps.append(m)

    b_v_full = np.concatenate([bv_of(h) for h in range(H)])
    host_bias = b_v_full @ wout.T + bout                # [D], exact
    return in_maps, host_bias


def kernel(**inputs):
    from concourse.bass_utils import run_bass_kernel_spmd
    self_mask = np.asarray(inputs["self_mask"], np.float32)
    use_mask = bool(np.any(self_mask))
    key = ("nc", use_mask)
    if key not in _CACHE:
        _CACHE[key] = build_nc(use_mask)
    nc = _CACHE[key]
    in_maps, host_bias = make_in_maps(
        inputs["mha_x"], self_mask, inputs["w_qkv"], inputs["b_qkv"],
        inputs["w_out"], inputs["b_out"], use_mask)
    res = run_bass_kernel_spmd(nc, in_maps, core_ids=list(range(NCORES)))
    acc = np.zeros((NBLK, 512, D), np.float32)
    for c in range(NCORES):
        # out is [blk, p, u, nb2, j]; token = 128u+p, feature = 512nb2+j
        arr = res.results[c]["out"].astype(np.float32)
        acc += arr.transpose(0, 2, 1, 3, 4).reshape(NBLK, 512, D)
    acc += host_bias[None, None, :]
    return acc.reshape(B, S, D)


# revision 29
# speedup vs baseline: 1.0149x; 1.0149x over previous
"""MultiHeadAttention forward on 8 Trainium2 NeuronCores (Bass/Tile).

Problem (hardcoded): B=2, S=2048, D=1024, H=16, HD=64.
  qkv = x @ w_qkv.T + b_qkv ; per-head attention with softmax(q k^T/8 + mask);
  out = values @ w_out.T + b_out.

Sharding: tensor-parallel over heads -- core c owns heads {2c, 2c+1}
(value dims 128c..128c+127).  Each core computes its 2 heads end-to-end and
a partial output projection; the host sums the 8 partials (bf16) and adds
the bias constant (b_out + b_v @ w_out.T, exact because softmax rows sum
to 1, and q.bk-type score shifts are softmax-invariant).

Device layout notes:
 - everything bf16 on the PE (same PE rate as f32r, half the DMA/SBUF).
 - scores are computed TRANSPOSED (S^T[k,tq] = K^T.T @ Q^T per head); the
   two heads' score matmuls hit disjoint PE row groups (partitions 0-63 /
   64-127) and run concurrently.
 - the exp stream is the serial bottleneck (128 x ~1.15us ACTIVATE =
   147us; ACT cost = (N+352)/1.2GHz, dtype-independent, and only ScalarE
   has activation LUTs), so everything is built to never stall it:
   * scores are emitted DEPTH=3 chunks ahead of AV in a flat
     (block, chunk) loop crossing block boundaries, and pt (probs) has
     an 8-deep pool: heavy Phase-A filler items make the in-order PE
     queue lag ~2-6us, and a 1-deep score pipeline lags with it.
   * the whole block epilogue (normalize + out-projection + out DMA) is
     queued as FILLER pulled during the NEXT block; emitted inline it
     head-blocks the next block's scores in the PE queue.
   * av is staged out of PSUM with cheap copies FIRST: the 2-buffer av
     pool is reused every block, and holding a buffer through the
     normalize chain stalls the next block's AV accumulation.
 - vext carries 32 ones columns so AV rows 64..95 hold the softmax
   denominator l; 32x32 DVE block transposes make l partition-parallel
   (DVE reciprocal costs ~6.5 cycles per FREE element -- 3.3us for
   [1,512] -- so the transposes are load-bearing), then a K=1 bf16 PE
   matmul broadcasts 1/l (bf16 here adds ~0.2% relative: fine vs the
   2e-2 gate; f32r matmuls measured ~700ns vs ~320ns bf16) and one DVE
   multiply (av_sb SBUF x bcp PSUM -- DVE allows ONE PSUM operand)
   writes valsT.
 - valsT is one [128, T] tile (head h on partitions 64h..64h+63) so the
   output projection is a single K=128 matmul per out-tile.
 - GpSimd is useless here: it has NO PSUM port (walrus rejects it) and
   is ~3x slower than DVE on small SBUF ops (1861ns for a [1,512] cast).
 - host pre-arranges x^T/wqk/wv/out into partition-major blocks so every
   DMA is a contiguous >=4KB run per partition (the naive (c p) j gather
   ran at ~73GB/s); even so a queue moves only ~137GB/s (descriptor
   setup bound), so the x stream rides the Activation HWDGE queue in
   parallel with the weight stream on the SP queue at startup.
 - ~7 dummy matmuls on a memset tile run first to flip the PE HAM clock
   gate (1.2 -> 2.4 GHz takes ~3.4us of sustained busy; >3.4us idle
   drops it back) while the first DMAs land.
 - the LAST block's epilogue has nothing to overlap: half its casts run
   on the idle ScalarE and its out DMA is split in two.
 - K=64 row-paired AV was analyzed useless (matmul time = N cycles
   regardless of K) and alternating PE tile configs inside one PSUM
   accumulation group crashes the NEFF execution on HW -- not used.
 - Custom-DVE ops (reciprocal_approx_fast) and GpSimd ucode ops
   (partition_broadcast) misbehave on this runtime -- plain ops only.
"""
import sys
if "/opt/trn_rl_repo" not in sys.path:
    sys.path.insert(0, "/opt/trn_rl_repo")
import numpy as np
from collections import deque

B, S, D, H = 2, 2048, 1024, 16
HD = D // H           # 64
NCORES = 8
T = B * S             # 4096 tokens
NB = S // 512         # 4 tq blocks per batch
NCH = S // 128        # 16 kpos chunks per batch
NBLK = B * NB         # 8 attention blocks total

_CACHE = {}


def build_nc(use_mask: bool, reps: int = 1):
    """Build + compile the per-core Bass program (SPMD-identical)."""
    import concourse.bacc as bacc
    import concourse.tile as tile
    from concourse import mybir

    f32 = mybir.dt.float32
    f32r = mybir.dt.float32r
    bf16 = mybir.dt.bfloat16
    EXP = mybir.ActivationFunctionType.Exp
    MULT = mybir.AluOpType.mult

    nc = bacc.Bacc("TRN2", target_bir_lowering=False, debug=False,
                   num_devices=NCORES)

    xTb = nc.dram_tensor("xTb", (8, 128, 8, 512), bf16, kind="ExternalInput")
    wqkT = nc.dram_tensor("wqkT", (128, 8, 256), bf16, kind="ExternalInput")
    bqk = nc.dram_tensor("bqk", (128, 2), f32, kind="ExternalInput")
    wvT = nc.dram_tensor("wvT", (128, 8, 128), bf16, kind="ExternalInput")
    woT = nc.dram_tensor("woT", (128, D), bf16, kind="ExternalInput")
    identb = nc.dram_tensor("identb", (128, 128), bf16, kind="ExternalInput")
    if use_mask:
        maskT = nc.dram_tensor("maskT", (B, S, S), f32r, kind="ExternalInput")
        ident = nc.dram_tensor("ident", (128, 128), f32r, kind="ExternalInput")
    out = nc.dram_tensor("out", (NBLK, 128, 4, 2, 512), bf16,
                         kind="ExternalOutput")

    with tile.TileContext(nc) as tc:
        with tc.tile_pool(name="sbp", bufs=1) as sbp, \
             tc.tile_pool(name="xtbp", bufs=3) as xtbp, \
             tc.tile_pool(name="ptp", bufs=8) as ptp, \
             tc.tile_pool(name="lrp", bufs=2) as lrp, \
             tc.tile_pool(name="otp", bufs=2) as otp, \
             tc.tile_pool(name="mkp", bufs=6) as mkp, \
             tc.tile_pool(name="mmp", bufs=2, space="PSUM") as mmp, \
             tc.tile_pool(name="scp", bufs=2, space="PSUM") as scp, \
             tc.tile_pool(name="avp", bufs=2, space="PSUM") as avp:

            # --- persistent SBUF tensors ---
            qkt = sbp.tile([128, 2, T], bf16, name="qkt")    # [feat,{q,k},tok]
            vext = sbp.tile([128, B, 2, NCH, HD + 32], bf16, name="vext")
            vT_sb = sbp.tile([128, T], bf16, name="vT_sb")   # [vfeat, tok]
            valsT = sbp.tile([128, T], bf16, name="valsT")   # [64h+d, tok]
            wqk_sb = sbp.tile([128, 8, 256], bf16, name="wqk_sb")
            wv_sb = sbp.tile([128, 8, 128], bf16, name="wv_sb")
            wo_sb = sbp.tile([128, D], bf16, name="wo_sb")
            bqk_sb = sbp.tile([128, 2], f32, name="bqk_sb")
            idb_sb = sbp.tile([128, 128], bf16, name="idb_sb")
            ones_sb = sbp.tile([65, 64], bf16, name="ones_sb")
            warm_sb = sbp.tile([128, 512], bf16, name="warm_sb")
            nc.vector.memset(ones_sb, 1.0)
            nc.vector.memset(warm_sb, 0.0)
            nc.vector.memset(vext[:, :, :, :, HD:HD + 32], 1.0)

            # --- PE HAM warm-up: ~7 dummy matmuls (~3us at the cold
            # 1.2 GHz clock) start flipping the clock gate to 2.4 GHz
            # while the input DMAs land; real qk work abuts and finishes
            # the 3.4us sustained-busy window.  One accumulation group.
            warm_ps = mmp.tile([128, 512], f32, tag="mm", name="warm_ps")
            for i in range(21):
                nc.tensor.matmul(warm_ps, warm_sb[:, 0:128], warm_sb,
                                 start=(i == 0), stop=(i == 20))
            # dummy reader (the BIR verifier rejects never-read PSUM)
            nc.vector.tensor_copy(warm_sb[0:1, 0:1], warm_ps[0:1, 0:1])

            # --- startup DMAs, ordered by first use: the first qk matmul
            # needs x(tb0) + wqk + bqk; everything else follows.
            # All DMAs on the SP queue: the Activation HWDGE queue was
            # measured ~3x slower (256KB in 7.4us), so parallel-queue
            # splitting loses.
            # q-weights first, then x, then k-weights: the first score
            # needs q for all 512 tokens but k only for chunk 0, and every
            # projection output needs ALL of x (the contraction is over
            # features) -- so the critical DMA bytes are wqk-q + x-full.
            xtb0 = xtbp.tile([128, 8, 512], bf16, tag="xtb", name="xtb_first")
            nc.sync.dma_start(wqk_sb[:, :, 0:128], wqkT[:, :, 0:128])
            for lo in (0, 4):
                nc.sync.dma_start(xtb0[:, lo:lo + 4, :],
                                  xTb[0, :, lo:lo + 4, :])
            nc.sync.dma_start(wqk_sb[:, :, 128:256], wqkT[:, :, 128:256])
            nc.sync.dma_start(bqk_sb, bqk[:, :])
            if use_mask:
                id_sb = sbp.tile([128, 128], f32r, name="id_sb")
                nc.sync.dma_start(id_sb, ident[:, :])
            nc.sync.dma_start(wv_sb, wvT[:, :, :])
            nc.sync.dma_start(idb_sb, identb[:, :])
            nc.sync.dma_start(wo_sb, woT[:, :])

            def phase_a_block(rep, b, tb, first=False):
                """Emit the list of closures for one 512-token projection
                block (tb in 0..7 global).  Returned items are emitted
                lazily as filler (or inline for tb0)."""
                items = []
                if first:
                    xtb_t = xtb0
                else:
                    xtb_t = xtbp.tile([128, 8, 512], bf16, tag="xtb",
                                      name=f"xtb_{rep}_{tb}")

                    def load(tb=tb, xtb_t=xtb_t):
                        nc.sync.dma_start(xtb_t, xTb[tb, :, :, :])
                    items.append(load)

                def qk_part(m, j0, j1):
                    acc = mmp.tile([128, 512], f32, tag="mm",
                                   name=f"qk_{rep}_{tb}_{m}_{j0}")
                    for c in range(8):
                        nc.tensor.matmul(
                            acc[:, 0:j1 - j0],
                            wqk_sb[:, c, 128 * m:128 * m + 128],
                            xtb_t[:, c, j0:j1], start=(c == 0), stop=(c == 7))
                    nc.vector.tensor_scalar_add(
                        qkt[:, m, 512 * tb + j0:512 * tb + j1],
                        acc[:, 0:j1 - j0], bqk_sb[:, m:m + 1])

                if first:
                    # the first score chunk needs q(all) + k(tokens 0:128)
                    # only -- emit those first so exp(0) fires ~2us after
                    # the x DMA lands; the k remainder follows.
                    items.append(lambda: qk_part(0, 0, 512))
                    items.append(lambda: qk_part(1, 0, 128))
                    items.append(lambda: qk_part(1, 128, 512))
                else:
                    items.append(lambda: qk_part(0, 0, 512))
                    items.append(lambda: qk_part(1, 0, 512))

                def vpass():
                    # v^T [vfeat, tok] with wv stationary (long streams,
                    # weight loads hidden), then cast to bf16 SBUF.
                    vacc = mmp.tile([128, 512], f32, tag="mm",
                                    name=f"vacc_{rep}_{tb}")
                    for c in range(8):
                        nc.tensor.matmul(
                            vacc, wv_sb[:, c, :], xtb_t[:, c, :],
                            start=(c == 0), stop=(c == 7))
                    nc.vector.tensor_copy(
                        vT_sb[:, 512 * tb:512 * tb + 512], vacc)
                items.append(vpass)

                for u in range(4):          # transpose to [kpos, feat] tiles
                    def vtrans(u=u):
                        t0g = 512 * tb + 128 * u
                        cc = (t0g % S) // 128
                        vtp = mmp.tile([128, 128], f32, tag="mm",
                                       name=f"vtp_{rep}_{tb}_{u}")
                        nc.tensor.matmul(vtp, vT_sb[:, t0g:t0g + 128], idb_sb,
                                         start=True, stop=True)
                        nc.vector.tensor_copy(
                            vext[:, b, :, cc, 0:HD],
                            vtp[:, :].rearrange("p (h d) -> p h d", h=2))
                    items.append(vtrans)
                return items

            def pull(filler, n):
                for _ in range(n):
                    if not filler:
                        return
                    filler.popleft()()

            def make_ctx(rep, i):
                b, tqb = divmod(i, NB)
                tq0 = S * b + 512 * tqb
                q_aps = [qkt[64 * h:64 * h + 64, 0, tq0:tq0 + 512]
                         for h in range(2)]
                avs = [avp.tile([96, 512], f32, tag="av",
                                name=f"av_{rep}_{b}_{tqb}_{h}")
                       for h in range(2)]
                return dict(b=b, tqb=tqb, tq0=tq0, q_aps=q_aps, avs=avs)

            def emit_scores(rep, ctx, c):
                b, tqb = ctx["b"], ctx["tqb"]
                sc = scp.tile([128, 1024], f32, tag="sc",
                              name=f"sc_{rep}_{b}_{tqb}_{c}")
                for h in range(2):
                    k_ap = qkt[64 * h:64 * h + 64, 1,
                               S * b + 128 * c:S * b + 128 * c + 128]
                    nc.tensor.matmul(
                        sc[:, 512 * h:512 * h + 512], k_ap, ctx["q_aps"][h],
                        start=True, stop=(not use_mask))
                if use_mask:
                    mt = mkp.tile([128, 512], f32r, tag="mk",
                                  name=f"mk_{rep}_{b}_{tqb}_{c}")
                    nc.sync.dma_start(
                        mt, maskT[b, 128 * c:128 * c + 128,
                                  512 * tqb:512 * tqb + 512])
                    for h in range(2):
                        nc.tensor.matmul(
                            sc[:, 512 * h:512 * h + 512], id_sb, mt,
                            start=False, stop=True)
                return sc

            def queue_block_epilogue(rep, ctx, filler, last=False):
                """Normalize + output projection + out DMA for a finished
                block, all as filler items pulled during the next block.
                For the LAST block (nothing left to overlap) half the
                PSUM->SBUF casts run on the now-idle ScalarE and the out
                DMA is split so transfers overlap the remaining casts."""
                b, tqb, tq0, avs = ctx["b"], ctx["tqb"], ctx["tq0"], ctx["avs"]
                blk = b * NB + tqb

                for h in range(2):
                    def norm(h=h):
                        # av rows 64..95 all hold l (32 ones columns in
                        # vext); 32x32 DVE block transposes make l
                        # partition-parallel because the DVE reciprocal
                        # costs ~6.5 cycles per FREE element (measured
                        # 3.3us for [1,512] -- the transposes are load-
                        # bearing), then a K=1 f32r PE matmul broadcasts
                        # 1/l across partitions for the DVE normalize.
                        # av is copied out of PSUM first so the (doubly-
                        # scarce) av buffer frees immediately: the NEXT
                        # block's AV accumulation reuses this buffer, and
                        # holding it head-blocks the in-order PE queue.
                        av = avs[h]
                        av_sb = lrp.tile([64, 512], f32, tag="avs",
                                         name=f"avs_{rep}_{b}_{h}_{tqb}")
                        nc.vector.tensor_copy(av_sb, av[0:64, :])
                        lt = lrp.tile([96, 512], f32, tag="lt",
                                      name=f"lt_{rep}_{b}_{h}_{tqb}")
                        nc.vector.transpose(lt[64:96, :], av[64:96, :])
                        lt3 = lt[64:96, :].rearrange(
                            "p (a b) -> p a b", b=32)[:, :, 0:1]
                        nc.vector.reciprocal(lt3, lt3)
                        rlrowf = lrp.tile([96, 512], f32, tag="rlrowf",
                                          name=f"rlrowf_{rep}_{b}_{h}_{tqb}")
                        nc.vector.transpose(rlrowf[64:96, :], lt[64:96, :])
                        # 1/l broadcast in bf16 (f32r matmuls measured
                        # ~700ns vs ~320ns bf16; 1/l at bf16 adds ~0.2%
                        # relative -- rel err stays well under 2e-2).  All
                        # on DVE: GpSimd is ~3x slower on these small ops
                        # (measured 1861ns for a [1,512] cast) and this
                        # chain gates the next block's outproj filler.
                        rlrow = lrp.tile([65, 512], bf16, tag="rlrow",
                                         name=f"rlrow_{rep}_{b}_{h}_{tqb}")
                        nc.vector.tensor_copy(rlrow[64:65, :],
                                              rlrowf[64:65, :])
                        bcp = mmp.tile([128, 512], f32, tag="mm",
                                       name=f"bcp_{rep}_{b}_{tqb}_{h}")
                        nc.tensor.matmul(
                            bcp[0:64, :], ones_sb[64:65, :],
                            rlrow[64:65, :], start=True, stop=True)
                        nc.vector.tensor_tensor(
                            valsT[64 * h:64 * h + 64, tq0:tq0 + 512],
                            av_sb, bcp[0:64, :], MULT)
                    filler.append(norm)

                ot = otp.tile([128, 4, 2, 512], bf16, tag="ot",
                              name=f"ot_{rep}_{b}_{tqb}")
                late = []
                for u in range(4):
                    for nb2 in range(2):
                        def op_item(u=u, nb2=nb2):
                            t0 = tq0 + 128 * u
                            op = mmp.tile([128, 512], f32, tag="mm",
                                          name=f"op_{rep}_{b}_{tqb}_{nb2}_{u}")
                            nc.tensor.matmul(
                                op, valsT[:, t0:t0 + 128],
                                wo_sb[:, 512 * nb2:512 * nb2 + 512],
                                start=True, stop=True)
                            if last and nb2 == 1:
                                nc.scalar.copy(ot[:, u, nb2, :], op)
                            else:
                                nc.vector.tensor_copy(ot[:, u, nb2, :], op)
                        late.append(op_item)
                    if last:
                        # fire each quarter's DMA as soon as its casts are
                        # done: the final 1MB at ~137GB/s is otherwise a
                        # ~7us serial tail.
                        def quarter_dma(u=u):
                            nc.sync.dma_start(out[blk, :, u:u + 1, :, :],
                                              ot[:, u:u + 1, :, :])
                        late.append(quarter_dma)

                if not last:
                    def out_dma():
                        nc.sync.dma_start(out[blk, :, :, :, :], ot)
                    late.append(out_dma)
                return late

            for rep in range(reps):
                filler = deque()
                # Only tb0 of Phase A is emitted directly: attention(b0,
                # tqb0) needs just the first k/v chunks, so tb1-3 stream in
                # as filler during its chunk loop (chunk 4c needs tb c,
                # pulled 2 items/chunk -> arrives just in time).  Batch 1's
                # blocks become filler for the later b0 attention blocks.
                for item in phase_a_block(rep, 0, 0, first=True):
                    item()
                for tb in range(1, 4):
                    # fire tb1-3's x DMAs now (right behind the weights in
                    # the queue; the relayout makes each a ~3us contiguous
                    # transfer) -- chunk 4c of b0 attention needs tb c.
                    items = phase_a_block(rep, 0, tb)
                    items.pop(0)()
                    filler.extend(items)

                # Flat (block, chunk) sequence with scores emitted DEPTH=3
                # chunks ahead of AV (across block boundaries): the exp
                # stream only depends on scores, and when heavy Phase-A
                # filler makes the in-order PE queue lag, a 1-deep score
                # pipeline lags with it and stalls ScalarE.  Depth 3 plus
                # the 8-deep pt pool decouples exp from PE lag.
                DEPTH = 3
                seq = [(i, c) for i in range(NBLK) for c in range(NCH)]
                ctxs = {}

                def get_ctx(i):
                    if i not in ctxs:
                        ctxs[i] = make_ctx(rep, i)
                    return ctxs[i]

                sc_q = deque()
                pending_epi = []
                for j in range(DEPTH):
                    sc_q.append(emit_scores(rep, get_ctx(seq[j][0]),
                                            seq[j][1]))
                for idx, (i, c) in enumerate(seq):
                    if idx + DEPTH < len(seq):
                        i2, c2 = seq[idx + DEPTH]
                        sc_q.append(emit_scores(rep, get_ctx(i2), c2))
                    if c == 5 and pending_epi:
                        # the previous block's out-projection waits on its
                        # normalize (a ~5us DVE chain); pulled any earlier
                        # it head-blocks this block's scores in the
                        # in-order PE queue and stalls the exp stream.
                        filler.extend(pending_epi)
                        pending_epi = []
                    pull(filler, 2)
                    ctx = get_ctx(i)
                    pt = ptp.tile([128, 1024], bf16, tag="pt",
                                  name=f"pt_{rep}_{i}_{c}")
                    nc.scalar.activation(pt, sc_q.popleft(), EXP)
                    for h in range(2):
                        nc.tensor.matmul(
                            ctx["avs"][h], vext[:, ctx["b"], h, c, :],
                            pt[:, 512 * h:512 * h + 512],
                            start=(c == 0), stop=(c == NCH - 1))
                    if c == NCH - 1:
                        pending_epi = queue_block_epilogue(
                            rep, ctx, filler, last=(i == NBLK - 1))
                        if ctx["b"] == 0:
                            filler.extend(
                                phase_a_block(rep, 1, 4 + ctx["tqb"]))
                filler.extend(pending_epi)
                while filler:
                    filler.popleft()()
    nc.compile()
    return nc


def make_in_maps(mha_x, self_mask, w_qkv, b_qkv, w_out, b_out, use_mask):
    """Host-side sharding / layout prep. Returns (in_maps, host_bias)."""
    import ml_dtypes
    bf = np.dtype(ml_dtypes.bfloat16)
    x = np.asarray(mha_x, np.float32).reshape(T, D)
    # x^T pre-arranged [tb, p, c, j] so each 512-token block is one
    # contiguous-per-partition DMA (8KB rows; the naive (c p) j gather
    # ran at ~73 GB/s, ~8us for 512KB)
    xTb_np = np.ascontiguousarray(
        x.T.reshape(8, 128, 8, 512).transpose(2, 1, 0, 3).astype(bf))
    scale = 1.0 / np.sqrt(np.float32(HD))               # 1/8
    wqkv = np.asarray(w_qkv, np.float32)
    bqkv = np.asarray(b_qkv, np.float32)
    wout = np.asarray(w_out, np.float32)
    bout = np.asarray(b_out, np.float32)

    # reference packs w_qkv rows as [H, (q,k,v), HD]: head h's q rows are
    # wqkv[192h:192h+64], k rows +64, v rows +128.
    wq_rows = lambda h: wqkv[192 * h:192 * h + 64, :]
    wk_rows = lambda h: wqkv[192 * h + 64:192 * h + 128, :]
    wv_rows = lambda h: wqkv[192 * h + 128:192 * h + 192, :]
    bq_of = lambda h: bqkv[192 * h:192 * h + 64]
    bk_of = lambda h: bqkv[192 * h + 64:192 * h + 128]
    bv_of = lambda h: bqkv[192 * h + 128:192 * h + 192]

    in_maps = []
    for c in range(NCORES):
        h0, h1 = 2 * c, 2 * c + 1
        wq = np.concatenate([wq_rows(h0), wq_rows(h1)], 0) * scale
        wk = np.concatenate([wk_rows(h0), wk_rows(h1)], 0)
        wv = np.concatenate([wv_rows(h0), wv_rows(h1)], 0)
        m = {
            "xTb": xTb_np,
            "wqkT": np.ascontiguousarray(
                np.concatenate([wq, wk], 0).T.reshape(
                    8, 128, 256).transpose(1, 0, 2).astype(bf)),
            "bqk": np.ascontiguousarray(
                np.stack([np.concatenate([bq_of(h0), bq_of(h1)]) * scale,
                          np.concatenate([bk_of(h0), bk_of(h1)])], 1)),
            "wvT": np.ascontiguousarray(
                wv.T.reshape(8, 128, 128).transpose(1, 0, 2).astype(bf)),
            "woT": np.ascontiguousarray(
                wout[:, 128 * c:128 * c + 128].T.astype(bf)),
            "identb": np.eye(128, dtype=np.float32).astype(bf),
        }
        if use_mask:
            m["maskT"] = np.ascontiguousarray(
                np.asarray(self_mask, np.float32).transpose(0, 2, 1))
            m["ident"] = np.eye(128, dtype=np.float32)
        in_maps.append(m)

    b_v_full = np.concatenate([bv_of(h) for h in range(H)])
    host_bias = b_v_full @ wout.T + bout                # [D], exact
    return in_maps, host_bias


def kernel(**inputs):
    from concourse.bass_utils import run_bass_kernel_spmd
    self_mask = np.asarray(inputs["self_mask"], np.float32)
    use_mask = bool(np.any(self_mask))
    key = ("nc", use_mask)
    if key not in _CACHE:
        _CACHE[key] = build_nc(use_mask)
    nc = _CACHE[key]
    in_maps, host_bias = make_in_maps(
        inputs["mha_x"], self_mask, inputs["w_qkv"], inputs["b_qkv"],
        inputs["w_out"], inputs["b_out"], use_mask)
    res = run_bass_kernel_spmd(nc, in_maps, core_ids=list(range(NCORES)))
    acc = np.zeros((NBLK, 512, D), np.float32)
    for c in range(NCORES):
        # out is [blk, p, u, nb2, j]; token = 128u+p, feature = 512nb2+j
        arr = res.results[c]["out"].astype(np.float32)
        acc += arr.transpose(0, 2, 1, 3, 4).reshape(NBLK, 512, D)
    acc += host_bias[None, None, :]
    return acc.reshape(B, S, D)


# revision 30
# speedup vs baseline: 1.0203x; 1.0054x over previous
"""MultiHeadAttention forward on 8 Trainium2 NeuronCores (Bass/Tile).

Problem (hardcoded): B=2, S=2048, D=1024, H=16, HD=64.
  qkv = x @ w_qkv.T + b_qkv ; per-head attention with softmax(q k^T/8 + mask);
  out = values @ w_out.T + b_out.

Sharding: tensor-parallel over heads -- core c owns heads {2c, 2c+1}
(value dims 128c..128c+127).  Each core computes its 2 heads end-to-end and
a partial output projection; the host sums the 8 partials (bf16) and adds
the bias constant (b_out + b_v @ w_out.T, exact because softmax rows sum
to 1, and q.bk-type score shifts are softmax-invariant).

Device layout notes:
 - everything bf16 on the PE (same PE rate as f32r, half the DMA/SBUF).
 - scores are computed TRANSPOSED (S^T[k,tq] = K^T.T @ Q^T per head); the
   two heads' score matmuls hit disjoint PE row groups (partitions 0-63 /
   64-127) and run concurrently.
 - the exp stream is the serial bottleneck (128 x ~1.15us ACTIVATE =
   147us; ACT cost = (N+352)/1.2GHz, dtype-independent, and only ScalarE
   has activation LUTs), so everything is built to never stall it:
   * scores are emitted DEPTH=3 chunks ahead of AV in a flat
     (block, chunk) loop crossing block boundaries, and pt (probs) has
     an 8-deep pool: heavy Phase-A filler items make the in-order PE
     queue lag ~2-6us, and a 1-deep score pipeline lags with it.
   * the whole block epilogue (normalize + out-projection + out DMA) is
     queued as FILLER pulled during the NEXT block; emitted inline it
     head-blocks the next block's scores in the PE queue.
   * av is staged out of PSUM with cheap copies FIRST: the 2-buffer av
     pool is reused every block, and holding a buffer through the
     normalize chain stalls the next block's AV accumulation.
 - vext carries 32 ones columns so AV rows 64..95 hold the softmax
   denominator l; 32x32 DVE block transposes make l partition-parallel
   (DVE reciprocal costs ~6.5 cycles per FREE element -- 3.3us for
   [1,512] -- so the transposes are load-bearing), then a K=1 bf16 PE
   matmul broadcasts 1/l (bf16 here adds ~0.2% relative: fine vs the
   2e-2 gate; f32r matmuls measured ~700ns vs ~320ns bf16) and one DVE
   multiply (av_sb SBUF x bcp PSUM -- DVE allows ONE PSUM operand)
   writes valsT.
 - valsT is one [128, T] tile (head h on partitions 64h..64h+63) so the
   output projection is a single K=128 matmul per out-tile.
 - GpSimd is useless here: it has NO PSUM port (walrus rejects it) and
   is ~3x slower than DVE on small SBUF ops (1861ns for a [1,512] cast).
 - host pre-arranges x^T/wqk/wv/out into partition-major blocks so every
   DMA is a contiguous >=4KB run per partition (the naive (c p) j gather
   ran at ~73GB/s); even so a queue moves only ~137GB/s (descriptor
   setup bound), so the x stream rides the Activation HWDGE queue in
   parallel with the weight stream on the SP queue at startup.
 - ~7 dummy matmuls on a memset tile run first to flip the PE HAM clock
   gate (1.2 -> 2.4 GHz takes ~3.4us of sustained busy; >3.4us idle
   drops it back) while the first DMAs land.
 - the LAST block's epilogue has nothing to overlap: half its casts run
   on the idle ScalarE and its out DMA is split in two.
 - K=64 row-paired AV was analyzed useless (matmul time = N cycles
   regardless of K) and alternating PE tile configs inside one PSUM
   accumulation group crashes the NEFF execution on HW -- not used.
 - Custom-DVE ops (reciprocal_approx_fast) and GpSimd ucode ops
   (partition_broadcast) misbehave on this runtime -- plain ops only.
"""
import sys
if "/opt/trn_rl_repo" not in sys.path:
    sys.path.insert(0, "/opt/trn_rl_repo")
import numpy as np
from collections import deque

B, S, D, H = 2, 2048, 1024, 16
HD = D // H           # 64
NCORES = 8
T = B * S             # 4096 tokens
NB = S // 512         # 4 tq blocks per batch
NCH = S // 128        # 16 kpos chunks per batch
NBLK = B * NB         # 8 attention blocks total

_CACHE = {}


def build_nc(use_mask: bool, reps: int = 1):
    """Build + compile the per-core Bass program (SPMD-identical)."""
    import concourse.bacc as bacc
    import concourse.tile as tile
    from concourse import mybir

    f32 = mybir.dt.float32
    f32r = mybir.dt.float32r
    bf16 = mybir.dt.bfloat16
    EXP = mybir.ActivationFunctionType.Exp
    MULT = mybir.AluOpType.mult

    nc = bacc.Bacc("TRN2", target_bir_lowering=False, debug=False,
                   num_devices=NCORES)

    xTb = nc.dram_tensor("xTb", (8, 128, 8, 512), bf16, kind="ExternalInput")
    wqkT = nc.dram_tensor("wqkT", (128, 8, 256), bf16, kind="ExternalInput")
    bqk = nc.dram_tensor("bqk", (128, 2), f32, kind="ExternalInput")
    wvT = nc.dram_tensor("wvT", (128, 8, 128), bf16, kind="ExternalInput")
    woT = nc.dram_tensor("woT", (128, D), bf16, kind="ExternalInput")
    identb = nc.dram_tensor("identb", (128, 128), bf16, kind="ExternalInput")
    if use_mask:
        maskT = nc.dram_tensor("maskT", (B, S, S), f32r, kind="ExternalInput")
        ident = nc.dram_tensor("ident", (128, 128), f32r, kind="ExternalInput")
    out = nc.dram_tensor("out", (NBLK, 128, 4, 2, 512), bf16,
                         kind="ExternalOutput")

    with tile.TileContext(nc) as tc:
        with tc.tile_pool(name="sbp", bufs=1) as sbp, \
             tc.tile_pool(name="xtbp", bufs=3) as xtbp, \
             tc.tile_pool(name="ptp", bufs=8) as ptp, \
             tc.tile_pool(name="lrp", bufs=2) as lrp, \
             tc.tile_pool(name="otp", bufs=2) as otp, \
             tc.tile_pool(name="mkp", bufs=6) as mkp, \
             tc.tile_pool(name="mmp", bufs=2, space="PSUM") as mmp, \
             tc.tile_pool(name="scp", bufs=2, space="PSUM") as scp, \
             tc.tile_pool(name="avp", bufs=2, space="PSUM") as avp:

            # --- persistent SBUF tensors ---
            qkt = sbp.tile([128, 2, T], bf16, name="qkt")    # [feat,{q,k},tok]
            vext = sbp.tile([128, B, 2, NCH, HD + 32], bf16, name="vext")
            vT_sb = sbp.tile([128, T], bf16, name="vT_sb")   # [vfeat, tok]
            valsT = sbp.tile([128, T], bf16, name="valsT")   # [64h+d, tok]
            wqk_sb = sbp.tile([128, 8, 256], bf16, name="wqk_sb")
            wv_sb = sbp.tile([128, 8, 128], bf16, name="wv_sb")
            wo_sb = sbp.tile([128, D], bf16, name="wo_sb")
            bqk_sb = sbp.tile([128, 2], f32, name="bqk_sb")
            idb_sb = sbp.tile([128, 128], bf16, name="idb_sb")
            ones_sb = sbp.tile([65, 64], bf16, name="ones_sb")
            warm_sb = sbp.tile([128, 512], bf16, name="warm_sb")
            nc.vector.memset(ones_sb, 1.0)
            nc.vector.memset(warm_sb, 0.0)
            nc.vector.memset(vext[:, :, :, :, HD:HD + 32], 1.0)

            # --- PE HAM warm-up: ~7 dummy matmuls (~3us at the cold
            # 1.2 GHz clock) start flipping the clock gate to 2.4 GHz
            # while the input DMAs land; real qk work abuts and finishes
            # the 3.4us sustained-busy window.  One accumulation group.
            warm_ps = mmp.tile([128, 512], f32, tag="mm", name="warm_ps")
            for i in range(21):
                nc.tensor.matmul(warm_ps, warm_sb[:, 0:128], warm_sb,
                                 start=(i == 0), stop=(i == 20))
            # dummy reader (the BIR verifier rejects never-read PSUM)
            nc.vector.tensor_copy(warm_sb[0:1, 0:1], warm_ps[0:1, 0:1])

            # --- startup DMAs, ordered by first use: the first qk matmul
            # needs x(tb0) + wqk + bqk; everything else follows.
            # All DMAs on the SP queue: the Activation HWDGE queue was
            # measured ~3x slower (256KB in 7.4us), so parallel-queue
            # splitting loses.
            # q-weights first, then x, then k-weights: the first score
            # needs q for all 512 tokens but k only for chunk 0, and every
            # projection output needs ALL of x (the contraction is over
            # features) -- so the critical DMA bytes are wqk-q + x-full.
            xtb0 = xtbp.tile([128, 8, 512], bf16, tag="xtb", name="xtb_first")
            nc.sync.dma_start(wqk_sb[:, :, 0:128], wqkT[:, :, 0:128])
            for lo in (0, 4):
                nc.sync.dma_start(xtb0[:, lo:lo + 4, :],
                                  xTb[0, :, lo:lo + 4, :])
            nc.sync.dma_start(wqk_sb[:, :, 128:256], wqkT[:, :, 128:256])
            nc.sync.dma_start(bqk_sb, bqk[:, :])
            if use_mask:
                id_sb = sbp.tile([128, 128], f32r, name="id_sb")
                nc.sync.dma_start(id_sb, ident[:, :])
            nc.sync.dma_start(wv_sb, wvT[:, :, :])
            nc.sync.dma_start(idb_sb, identb[:, :])
            nc.sync.dma_start(wo_sb, woT[:, :])

            def phase_a_block(rep, b, tb, first=False):
                """Emit the list of closures for one 512-token projection
                block (tb in 0..7 global).  Returned items are emitted
                lazily as filler (or inline for tb0)."""
                items = []
                if first:
                    xtb_t = xtb0
                else:
                    xtb_t = xtbp.tile([128, 8, 512], bf16, tag="xtb",
                                      name=f"xtb_{rep}_{tb}")

                    def load(tb=tb, xtb_t=xtb_t):
                        nc.sync.dma_start(xtb_t, xTb[tb, :, :, :])
                    items.append(load)

                def qk_part(m, j0, j1):
                    acc = mmp.tile([128, 512], f32, tag="mm",
                                   name=f"qk_{rep}_{tb}_{m}_{j0}")
                    for c in range(8):
                        nc.tensor.matmul(
                            acc[:, 0:j1 - j0],
                            wqk_sb[:, c, 128 * m:128 * m + 128],
                            xtb_t[:, c, j0:j1], start=(c == 0), stop=(c == 7))
                    nc.vector.tensor_scalar_add(
                        qkt[:, m, 512 * tb + j0:512 * tb + j1],
                        acc[:, 0:j1 - j0], bqk_sb[:, m:m + 1])

                if first:
                    # the first score chunk needs q(all) + k(tokens 0:128)
                    # only -- emit those first so exp(0) fires ~2us after
                    # the x DMA lands; the k remainder follows.
                    items.append(lambda: qk_part(0, 0, 512))
                    items.append(lambda: qk_part(1, 0, 128))
                    items.append(lambda: qk_part(1, 128, 512))
                else:
                    items.append(lambda: qk_part(0, 0, 512))
                    items.append(lambda: qk_part(1, 0, 512))

                def vpass():
                    # v^T [vfeat, tok] with wv stationary (long streams,
                    # weight loads hidden), then cast to bf16 SBUF.
                    vacc = mmp.tile([128, 512], f32, tag="mm",
                                    name=f"vacc_{rep}_{tb}")
                    for c in range(8):
                        nc.tensor.matmul(
                            vacc, wv_sb[:, c, :], xtb_t[:, c, :],
                            start=(c == 0), stop=(c == 7))
                    nc.vector.tensor_copy(
                        vT_sb[:, 512 * tb:512 * tb + 512], vacc)
                items.append(vpass)

                for u in range(4):          # transpose to [kpos, feat] tiles
                    def vtrans(u=u):
                        t0g = 512 * tb + 128 * u
                        cc = (t0g % S) // 128
                        vtp = mmp.tile([128, 128], f32, tag="mm",
                                       name=f"vtp_{rep}_{tb}_{u}")
                        nc.tensor.matmul(vtp, vT_sb[:, t0g:t0g + 128], idb_sb,
                                         start=True, stop=True)
                        nc.vector.tensor_copy(
                            vext[:, b, :, cc, 0:HD],
                            vtp[:, :].rearrange("p (h d) -> p h d", h=2))
                    items.append(vtrans)
                return items

            def pull(filler, n):
                for _ in range(n):
                    if not filler:
                        return
                    filler.popleft()()

            def make_ctx(rep, i):
                b, tqb = divmod(i, NB)
                tq0 = S * b + 512 * tqb
                q_aps = [qkt[64 * h:64 * h + 64, 0, tq0:tq0 + 512]
                         for h in range(2)]
                avs = [avp.tile([96, 512], f32, tag="av",
                                name=f"av_{rep}_{b}_{tqb}_{h}")
                       for h in range(2)]
                return dict(b=b, tqb=tqb, tq0=tq0, q_aps=q_aps, avs=avs)

            def emit_scores(rep, ctx, c):
                b, tqb = ctx["b"], ctx["tqb"]
                sc = scp.tile([128, 1024], f32, tag="sc",
                              name=f"sc_{rep}_{b}_{tqb}_{c}")
                for h in range(2):
                    k_ap = qkt[64 * h:64 * h + 64, 1,
                               S * b + 128 * c:S * b + 128 * c + 128]
                    nc.tensor.matmul(
                        sc[:, 512 * h:512 * h + 512], k_ap, ctx["q_aps"][h],
                        start=True, stop=(not use_mask))
                if use_mask:
                    mt = mkp.tile([128, 512], f32r, tag="mk",
                                  name=f"mk_{rep}_{b}_{tqb}_{c}")
                    nc.sync.dma_start(
                        mt, maskT[b, 128 * c:128 * c + 128,
                                  512 * tqb:512 * tqb + 512])
                    for h in range(2):
                        nc.tensor.matmul(
                            sc[:, 512 * h:512 * h + 512], id_sb, mt,
                            start=False, stop=True)
                return sc

            def queue_block_epilogue(rep, ctx, filler, last=False):
                """Normalize + output projection + out DMA for a finished
                block, all as filler items pulled during the next block.
                For the LAST block (nothing left to overlap) half the
                PSUM->SBUF casts run on the now-idle ScalarE and the out
                DMA is split so transfers overlap the remaining casts."""
                b, tqb, tq0, avs = ctx["b"], ctx["tqb"], ctx["tq0"], ctx["avs"]
                blk = b * NB + tqb

                if last:
                    # keep the PE clock warm through the ~5us DVE dance
                    # (a >3.4us PE idle drops the HAM gate to 1.2 GHz and
                    # the out-projection then runs at half clock)
                    def tail_warm():
                        wps = mmp.tile([128, 512], f32, tag="mm",
                                       name=f"tailwarm_{rep}")
                        for i in range(10):
                            nc.tensor.matmul(wps, warm_sb[:, 0:128], warm_sb,
                                             start=(i == 0), stop=(i == 9))
                        nc.vector.tensor_copy(warm_sb[0:1, 1:2],
                                              wps[0:1, 0:1])
                    filler.append(tail_warm)

                for h in range(2):
                    def norm(h=h):
                        # av rows 64..95 all hold l (32 ones columns in
                        # vext); 32x32 DVE block transposes make l
                        # partition-parallel because the DVE reciprocal
                        # costs ~6.5 cycles per FREE element (measured
                        # 3.3us for [1,512] -- the transposes are load-
                        # bearing), then a K=1 f32r PE matmul broadcasts
                        # 1/l across partitions for the DVE normalize.
                        # av is copied out of PSUM first so the (doubly-
                        # scarce) av buffer frees immediately: the NEXT
                        # block's AV accumulation reuses this buffer, and
                        # holding it head-blocks the in-order PE queue.
                        av = avs[h]
                        if not last:
                            av_sb = lrp.tile([64, 512], f32, tag="avs",
                                             name=f"avs_{rep}_{b}_{h}_{tqb}")
                            nc.vector.tensor_copy(av_sb, av[0:64, :])
                        lt = lrp.tile([96, 512], f32, tag="lt",
                                      name=f"lt_{rep}_{b}_{h}_{tqb}")
                        nc.vector.transpose(lt[64:96, :], av[64:96, :])
                        lt3 = lt[64:96, :].rearrange(
                            "p (a b) -> p a b", b=32)[:, :, 0:1]
                        nc.vector.reciprocal(lt3, lt3)
                        rlrowf = lrp.tile([96, 512], f32, tag="rlrowf",
                                          name=f"rlrowf_{rep}_{b}_{h}_{tqb}")
                        nc.vector.transpose(rlrowf[64:96, :], lt[64:96, :])
                        # 1/l broadcast in bf16 (f32r matmuls measured
                        # ~700ns vs ~320ns bf16; 1/l at bf16 adds ~0.2%
                        # relative -- rel err stays well under 2e-2).  All
                        # on DVE: GpSimd is ~3x slower on these small ops
                        # (measured 1861ns for a [1,512] cast) and this
                        # chain gates the next block's outproj filler.
                        rlrow = lrp.tile([65, 512], bf16, tag="rlrow",
                                         name=f"rlrow_{rep}_{b}_{h}_{tqb}")
                        nc.vector.tensor_copy(rlrow[64:65, :],
                                              rlrowf[64:65, :])
                        bcp = mmp.tile([128, 512], f32, tag="mm",
                                       name=f"bcp_{rep}_{b}_{tqb}_{h}")
                        nc.tensor.matmul(
                            bcp[0:64, :], ones_sb[64:65, :],
                            rlrow[64:65, :], start=True, stop=True)
                        if last:
                            # nothing left to overlap: stage the broadcast
                            # via the idle ScalarE and read av straight
                            # from PSUM (one-PSUM-operand rule), skipping
                            # the av_sb copy on the serial tail chain.
                            bcs = lrp.tile([64, 512], f32, tag="avs",
                                           name=f"bcs_{rep}_{b}_{h}_{tqb}")
                            nc.scalar.copy(bcs, bcp[0:64, :])
                            nc.vector.tensor_tensor(
                                valsT[64 * h:64 * h + 64, tq0:tq0 + 512],
                                bcs, av[0:64, :], MULT)
                        else:
                            nc.vector.tensor_tensor(
                                valsT[64 * h:64 * h + 64, tq0:tq0 + 512],
                                av_sb, bcp[0:64, :], MULT)
                    filler.append(norm)

                ot = otp.tile([128, 4, 2, 512], bf16, tag="ot",
                              name=f"ot_{rep}_{b}_{tqb}")
                late = []
                for u in range(4):
                    for nb2 in range(2):
                        def op_item(u=u, nb2=nb2):
                            t0 = tq0 + 128 * u
                            op = mmp.tile([128, 512], f32, tag="mm",
                                          name=f"op_{rep}_{b}_{tqb}_{nb2}_{u}")
                            nc.tensor.matmul(
                                op, valsT[:, t0:t0 + 128],
                                wo_sb[:, 512 * nb2:512 * nb2 + 512],
                                start=True, stop=True)
                            if last and nb2 == 1:
                                nc.scalar.copy(ot[:, u, nb2, :], op)
                            else:
                                nc.vector.tensor_copy(ot[:, u, nb2, :], op)
                        late.append(op_item)
                    if last:
                        # fire each quarter's DMA as soon as its casts are
                        # done: the final 1MB at ~137GB/s is otherwise a
                        # ~7us serial tail.
                        def quarter_dma(u=u):
                            nc.sync.dma_start(out[blk, :, u:u + 1, :, :],
                                              ot[:, u:u + 1, :, :])
                        late.append(quarter_dma)

                if not last:
                    def out_dma():
                        nc.sync.dma_start(out[blk, :, :, :, :], ot)
                    late.append(out_dma)
                return late

            for rep in range(reps):
                filler = deque()
                # Only tb0 of Phase A is emitted directly: attention(b0,
                # tqb0) needs just the first k/v chunks, so tb1-3 stream in
                # as filler during its chunk loop (chunk 4c needs tb c,
                # pulled 2 items/chunk -> arrives just in time).  Batch 1's
                # blocks become filler for the later b0 attention blocks.
                for item in phase_a_block(rep, 0, 0, first=True):
                    item()
                for tb in range(1, 4):
                    # fire tb1-3's x DMAs now (right behind the weights in
                    # the queue; the relayout makes each a ~3us contiguous
                    # transfer) -- chunk 4c of b0 attention needs tb c.
                    items = phase_a_block(rep, 0, tb)
                    items.pop(0)()
                    filler.extend(items)

                # Flat (block, chunk) sequence with scores emitted DEPTH=3
                # chunks ahead of AV (across block boundaries): the exp
                # stream only depends on scores, and when heavy Phase-A
                # filler makes the in-order PE queue lag, a 1-deep score
                # pipeline lags with it and stalls ScalarE.  Depth 3 plus
                # the 8-deep pt pool decouples exp from PE lag.
                DEPTH = 3
                seq = [(i, c) for i in range(NBLK) for c in range(NCH)]
                ctxs = {}

                def get_ctx(i):
                    if i not in ctxs:
                        ctxs[i] = make_ctx(rep, i)
                    return ctxs[i]

                sc_q = deque()
                pending_epi = []
                for j in range(DEPTH):
                    sc_q.append(emit_scores(rep, get_ctx(seq[j][0]),
                                            seq[j][1]))
                for idx, (i, c) in enumerate(seq):
                    if idx + DEPTH < len(seq):
                        i2, c2 = seq[idx + DEPTH]
                        sc_q.append(emit_scores(rep, get_ctx(i2), c2))
                    if c == 5 and pending_epi:
                        # the previous block's out-projection waits on its
                        # normalize (a ~5us DVE chain); pulled any earlier
                        # it head-blocks this block's scores in the
                        # in-order PE queue and stalls the exp stream.
                        filler.extend(pending_epi)
                        pending_epi = []
                    pull(filler, 2)
                    ctx = get_ctx(i)
                    pt = ptp.tile([128, 1024], bf16, tag="pt",
                                  name=f"pt_{rep}_{i}_{c}")
                    nc.scalar.activation(pt, sc_q.popleft(), EXP)
                    for h in range(2):
                        nc.tensor.matmul(
                            ctx["avs"][h], vext[:, ctx["b"], h, c, :],
                            pt[:, 512 * h:512 * h + 512],
                            start=(c == 0), stop=(c == NCH - 1))
                    if c == NCH - 1:
                        pending_epi = queue_block_epilogue(
                            rep, ctx, filler, last=(i == NBLK - 1))
                        if ctx["b"] == 0:
                            filler.extend(
                                phase_a_block(rep, 1, 4 + ctx["tqb"]))
                filler.extend(pending_epi)
                while filler:
                    filler.popleft()()
    nc.compile()
    return nc


def make_in_maps(mha_x, self_mask, w_qkv, b_qkv, w_out, b_out, use_mask):
    """Host-side sharding / layout prep. Returns (in_maps, host_bias)."""
    import ml_dtypes
    bf = np.dtype(ml_dtypes.bfloat16)
    x = np.asarray(mha_x, np.float32).reshape(T, D)
    # x^T pre-arranged [tb, p, c, j] so each 512-token block is one
    # contiguous-per-partition DMA (8KB rows; the naive (c p) j gather
    # ran at ~73 GB/s, ~8us for 512KB)
    xTb_np = np.ascontiguousarray(
        x.T.reshape(8, 128, 8, 512).transpose(2, 1, 0, 3).astype(bf))
    scale = 1.0 / np.sqrt(np.float32(HD))               # 1/8
    wqkv = np.asarray(w_qkv, np.float32)
    bqkv = np.asarray(b_qkv, np.float32)
    wout = np.asarray(w_out, np.float32)
    bout = np.asarray(b_out, np.float32)

    # reference packs w_qkv rows as [H, (q,k,v), HD]: head h's q rows are
    # wqkv[192h:192h+64], k rows +64, v rows +128.
    wq_rows = lambda h: wqkv[192 * h:192 * h + 64, :]
    wk_rows = lambda h: wqkv[192 * h + 64:192 * h + 128, :]
    wv_rows = lambda h: wqkv[192 * h + 128:192 * h + 192, :]
    bq_of = lambda h: bqkv[192 * h:192 * h + 64]
    bk_of = lambda h: bqkv[192 * h + 64:192 * h + 128]
    bv_of = lambda h: bqkv[192 * h + 128:192 * h + 192]

    in_maps = []
    for c in range(NCORES):
        h0, h1 = 2 * c, 2 * c + 1
        wq = np.concatenate([wq_rows(h0), wq_rows(h1)], 0) * scale
        wk = np.concatenate([wk_rows(h0), wk_rows(h1)], 0)
        wv = np.concatenate([wv_rows(h0), wv_rows(h1)], 0)
        m = {
            "xTb": xTb_np,
            "wqkT": np.ascontiguousarray(
                np.concatenate([wq, wk], 0).T.reshape(
                    8, 128, 256).transpose(1, 0, 2).astype(bf)),
            "bqk": np.ascontiguousarray(
                np.stack([np.concatenate([bq_of(h0), bq_of(h1)]) * scale,
                          np.concatenate([bk_of(h0), bk_of(h1)])], 1)),
            "wvT": np.ascontiguousarray(
                wv.T.reshape(8, 128, 128).transpose(1, 0, 2).astype(bf)),
            "woT": np.ascontiguousarray(
                wout[:, 128 * c:128 * c + 128].T.astype(bf)),
            "identb": np.eye(128, dtype=np.float32).astype(bf),
        }
        if use_mask:
            m["maskT"] = np.ascontiguousarray(
                np.asarray(self_mask, np.float32).transpose(0, 2, 1))
            m["ident"] = np.eye(128, dtype=np.float32)
        in_maps.append(m)

    b_v_full = np.concatenate([bv_of(h) for h in range(H)])
    host_bias = b_v_full @ wout.T + bout                # [D], exact
    return in_maps, host_bias


def kernel(**inputs):
    from concourse.bass_utils import run_bass_kernel_spmd
    self_mask = np.asarray(inputs["self_mask"], np.float32)
    use_mask = bool(np.any(self_mask))
    key = ("nc", use_mask)
    if key not in _CACHE:
        _CACHE[key] = build_nc(use_mask)
    nc = _CACHE[key]
    in_maps, host_bias = make_in_maps(
        inputs["mha_x"], self_mask, inputs["w_qkv"], inputs["b_qkv"],
        inputs["w_out"], inputs["b_out"], use_mask)
    res = run_bass_kernel_spmd(nc, in_maps, core_ids=list(range(NCORES)))
    acc = np.zeros((NBLK, 512, D), np.float32)
    for c in range(NCORES):
        # out is [blk, p, u, nb2, j]; token = 128u+p, feature = 512nb2+j
        arr = res.results[c]["out"].astype(np.float32)
        acc += arr.transpose(0, 2, 1, 3, 4).reshape(NBLK, 512, D)
    acc += host_bias[None, None, :]
    return acc.reshape(B, S, D)


# revision 31
# speedup vs baseline: 1.0543x; 1.0332x over previous
"""MultiHeadAttention forward on 8 Trainium2 NeuronCores (Bass/Tile).

Problem (hardcoded): B=2, S=2048, D=1024, H=16, HD=64.
  qkv = x @ w_qkv.T + b_qkv ; per-head attention with softmax(q k^T/8 + mask);
  out = values @ w_out.T + b_out.

Sharding: tensor-parallel over heads -- core c owns heads {2c, 2c+1}
(value dims 128c..128c+127).  Each core computes its 2 heads end-to-end and
a partial output projection; the host sums the 8 partials (bf16) and adds
the bias constant (b_out + b_v @ w_out.T, exact because softmax rows sum
to 1, and q.bk-type score shifts are softmax-invariant).

Device layout notes:
 - everything bf16 on the PE (same PE rate as f32r, half the DMA/SBUF).
 - scores are computed TRANSPOSED (S^T[k,tq] = K^T.T @ Q^T per head); the
   two heads' score matmuls hit disjoint PE row groups (partitions 0-63 /
   64-127) and run concurrently.
 - the exp stream is the serial bottleneck (128 x ~1.15us ACTIVATE =
   147us; ACT cost = (N+352)/1.2GHz, dtype-independent, and only ScalarE
   has activation LUTs), so everything is built to never stall it:
   * scores are emitted DEPTH=3 chunks ahead of AV in a flat
     (block, chunk) loop crossing block boundaries, and pt (probs) has
     an 8-deep pool: heavy Phase-A filler items make the in-order PE
     queue lag ~2-6us, and a 1-deep score pipeline lags with it.
   * the whole block epilogue (normalize + out-projection + out DMA) is
     queued as FILLER pulled during the NEXT block; emitted inline it
     head-blocks the next block's scores in the PE queue.
   * av is staged out of PSUM with cheap copies FIRST: the 2-buffer av
     pool is reused every block, and holding a buffer through the
     normalize chain stalls the next block's AV accumulation.
 - vext carries 32 ones columns so AV rows 64..95 hold the softmax
   denominator l; 32x32 DVE block transposes make l partition-parallel
   (DVE reciprocal costs ~6.5 cycles per FREE element -- 3.3us for
   [1,512] -- so the transposes are load-bearing), then a K=1 bf16 PE
   matmul broadcasts 1/l (bf16 here adds ~0.2% relative: fine vs the
   2e-2 gate; f32r matmuls measured ~700ns vs ~320ns bf16) and one DVE
   multiply (av_sb SBUF x bcp PSUM -- DVE allows ONE PSUM operand)
   writes valsT.
 - valsT is one [128, T] tile (head h on partitions 64h..64h+63) so the
   output projection is a single K=128 matmul per out-tile.
 - GpSimd is useless here: it has NO PSUM port (walrus rejects it) and
   is ~3x slower than DVE on small SBUF ops (1861ns for a [1,512] cast).
 - host pre-arranges x^T/wqk/wv/out into partition-major blocks so every
   DMA is a contiguous >=4KB run per partition (the naive (c p) j gather
   ran at ~73GB/s); even so a queue moves only ~137GB/s (descriptor
   setup bound), so the x stream rides the Activation HWDGE queue in
   parallel with the weight stream on the SP queue at startup.
 - ~7 dummy matmuls on a memset tile run first to flip the PE HAM clock
   gate (1.2 -> 2.4 GHz takes ~3.4us of sustained busy; >3.4us idle
   drops it back) while the first DMAs land.
 - the LAST block's epilogue has nothing to overlap: half its casts run
   on the idle ScalarE and its out DMA is split in two.
 - K=64 row-paired AV was analyzed useless (matmul time = N cycles
   regardless of K) and alternating PE tile configs inside one PSUM
   accumulation group crashes the NEFF execution on HW -- not used.
 - Custom-DVE ops (reciprocal_approx_fast) and GpSimd ucode ops
   (partition_broadcast) misbehave on this runtime -- plain ops only.
"""
import sys
if "/opt/trn_rl_repo" not in sys.path:
    sys.path.insert(0, "/opt/trn_rl_repo")
import numpy as np
from collections import deque

B, S, D, H = 2, 2048, 1024, 16
HD = D // H           # 64
NCORES = 8
T = B * S             # 4096 tokens
NB = S // 512         # 4 tq blocks per batch
NCH = S // 128        # 16 kpos chunks per batch
NBLK = B * NB         # 8 attention blocks total

_CACHE = {}


def build_nc(use_mask: bool, reps: int = 1):
    """Build + compile the per-core Bass program (SPMD-identical)."""
    import concourse.bacc as bacc
    import concourse.tile as tile
    from concourse import mybir

    f32 = mybir.dt.float32
    f32r = mybir.dt.float32r
    bf16 = mybir.dt.bfloat16
    EXP = mybir.ActivationFunctionType.Exp
    MULT = mybir.AluOpType.mult

    nc = bacc.Bacc("TRN2", target_bir_lowering=False, debug=False,
                   num_devices=NCORES)

    xTb = nc.dram_tensor("xTb", (8, 128, 8, 512), bf16, kind="ExternalInput")
    wqkT = nc.dram_tensor("wqkT", (128, 8, 256), bf16, kind="ExternalInput")
    bqk = nc.dram_tensor("bqk", (128, 2), f32, kind="ExternalInput")
    wvT = nc.dram_tensor("wvT", (128, 8, 128), bf16, kind="ExternalInput")
    woT = nc.dram_tensor("woT", (128, D), bf16, kind="ExternalInput")
    identb = nc.dram_tensor("identb", (128, 128), bf16, kind="ExternalInput")
    if use_mask:
        maskT = nc.dram_tensor("maskT", (B, S, S), f32r, kind="ExternalInput")
        ident = nc.dram_tensor("ident", (128, 128), f32r, kind="ExternalInput")
    out = nc.dram_tensor("out", (NBLK, 128, 4, 2, 512), bf16,
                         kind="ExternalOutput")

    with tile.TileContext(nc) as tc:
        with tc.tile_pool(name="sbp", bufs=1) as sbp, \
             tc.tile_pool(name="xtbp", bufs=3) as xtbp, \
             tc.tile_pool(name="ptp", bufs=8) as ptp, \
             tc.tile_pool(name="lrp", bufs=2) as lrp, \
             tc.tile_pool(name="otp", bufs=2) as otp, \
             tc.tile_pool(name="mkp", bufs=6) as mkp, \
             tc.tile_pool(name="mmp", bufs=2, space="PSUM") as mmp, \
             tc.tile_pool(name="scp", bufs=2, space="PSUM") as scp, \
             tc.tile_pool(name="avp", bufs=2, space="PSUM") as avp:

            # --- persistent SBUF tensors ---
            qkt = sbp.tile([128, 2, T], bf16, name="qkt")    # [feat,{q,k},tok]
            vext = sbp.tile([128, B, 2, NCH, HD + 32], bf16, name="vext")
            vT_sb = sbp.tile([128, T], bf16, name="vT_sb")   # [vfeat, tok]
            valsT = sbp.tile([128, T], bf16, name="valsT")   # [64h+d, tok]
            wqk_sb = sbp.tile([128, 8, 256], bf16, name="wqk_sb")
            wv_sb = sbp.tile([128, 8, 128], bf16, name="wv_sb")
            wo_sb = sbp.tile([128, D], bf16, name="wo_sb")
            bqk_sb = sbp.tile([128, 2], f32, name="bqk_sb")
            idb_sb = sbp.tile([128, 128], bf16, name="idb_sb")
            ones_sb = sbp.tile([65, 64], bf16, name="ones_sb")
            warm_sb = sbp.tile([128, 512], bf16, name="warm_sb")
            nc.vector.memset(ones_sb, 1.0)
            nc.vector.memset(warm_sb, 0.0)
            nc.vector.memset(vext[:, :, :, :, HD:HD + 32], 1.0)

            # --- PE HAM warm-up: ~7 dummy matmuls (~3us at the cold
            # 1.2 GHz clock) start flipping the clock gate to 2.4 GHz
            # while the input DMAs land; real qk work abuts and finishes
            # the 3.4us sustained-busy window.  One accumulation group.
            warm_ps = mmp.tile([128, 512], f32, tag="mm", name="warm_ps")
            for i in range(21):
                nc.tensor.matmul(warm_ps, warm_sb[:, 0:128], warm_sb,
                                 start=(i == 0), stop=(i == 20))
            # dummy reader (the BIR verifier rejects never-read PSUM)
            nc.vector.tensor_copy(warm_sb[0:1, 0:1], warm_ps[0:1, 0:1])

            # --- startup DMAs, ordered by first use: the first qk matmul
            # needs x(tb0) + wqk + bqk; everything else follows.
            # All DMAs on the SP queue: the Activation HWDGE queue was
            # measured ~3x slower (256KB in 7.4us), so parallel-queue
            # splitting loses.
            # q-weights first, then x, then k-weights: the first score
            # needs q for all 512 tokens but k only for chunk 0, and every
            # projection output needs ALL of x (the contraction is over
            # features) -- so the critical DMA bytes are wqk-q + x-full.
            xtb0 = xtbp.tile([128, 8, 512], bf16, tag="xtb", name="xtb_first")
            nc.sync.dma_start(wqk_sb[:, :, 0:128], wqkT[:, :, 0:128])
            for lo in (0, 4):
                nc.sync.dma_start(xtb0[:, lo:lo + 4, :],
                                  xTb[0, :, lo:lo + 4, :])
            nc.sync.dma_start(wqk_sb[:, :, 128:256], wqkT[:, :, 128:256])
            nc.sync.dma_start(bqk_sb, bqk[:, :])
            if use_mask:
                id_sb = sbp.tile([128, 128], f32r, name="id_sb")
                nc.sync.dma_start(id_sb, ident[:, :])
            nc.sync.dma_start(wv_sb, wvT[:, :, :])
            nc.sync.dma_start(idb_sb, identb[:, :])
            nc.sync.dma_start(wo_sb, woT[:, :])

            def phase_a_block(rep, b, tb, first=False):
                """Emit the list of closures for one 512-token projection
                block (tb in 0..7 global).  Returned items are emitted
                lazily as filler (or inline for tb0)."""
                items = []
                if first:
                    xtb_t = xtb0
                else:
                    xtb_t = xtbp.tile([128, 8, 512], bf16, tag="xtb",
                                      name=f"xtb_{rep}_{tb}")

                    def load(tb=tb, xtb_t=xtb_t):
                        nc.sync.dma_start(xtb_t, xTb[tb, :, :, :])
                    items.append(load)

                def qk_part(m, j0, j1):
                    acc = mmp.tile([128, 512], f32, tag="mm",
                                   name=f"qk_{rep}_{tb}_{m}_{j0}")
                    for c in range(8):
                        nc.tensor.matmul(
                            acc[:, 0:j1 - j0],
                            wqk_sb[:, c, 128 * m:128 * m + 128],
                            xtb_t[:, c, j0:j1], start=(c == 0), stop=(c == 7))
                    nc.vector.tensor_scalar_add(
                        qkt[:, m, 512 * tb + j0:512 * tb + j1],
                        acc[:, 0:j1 - j0], bqk_sb[:, m:m + 1])

                if first:
                    # the first score chunk needs q(all) + k(tokens 0:128)
                    # only -- emit those first so exp(0) fires ~2us after
                    # the x DMA lands; the k remainder follows.
                    items.append(lambda: qk_part(0, 0, 512))
                    items.append(lambda: qk_part(1, 0, 128))
                    items.append(lambda: qk_part(1, 128, 512))
                else:
                    items.append(lambda: qk_part(0, 0, 512))
                    items.append(lambda: qk_part(1, 0, 512))

                def vpass():
                    # v^T [vfeat, tok] with wv stationary (long streams,
                    # weight loads hidden), then cast to bf16 SBUF.
                    vacc = mmp.tile([128, 512], f32, tag="mm",
                                    name=f"vacc_{rep}_{tb}")
                    for c in range(8):
                        nc.tensor.matmul(
                            vacc, wv_sb[:, c, :], xtb_t[:, c, :],
                            start=(c == 0), stop=(c == 7))
                    nc.vector.tensor_copy(
                        vT_sb[:, 512 * tb:512 * tb + 512], vacc)
                items.append(vpass)

                for u in range(2):          # transpose to [kpos, feat] tiles
                    def vtrans(u=u):
                        # two 128-token chunks per item: halves the matmul,
                        # ldweights and DVE-copy counts vs per-chunk items
                        t0g = 512 * tb + 256 * u
                        cc = (t0g % S) // 128
                        vtp = mmp.tile([128, 512], f32, tag="mm",
                                       name=f"vtp_{rep}_{tb}_{u}")
                        for k in range(2):
                            nc.tensor.matmul(
                                vtp[:, 128 * k:128 * k + 128],
                                vT_sb[:, t0g + 128 * k:t0g + 128 * k + 128],
                                idb_sb, start=True, stop=True)
                        nc.vector.tensor_copy(
                            vext[:, b, :, cc:cc + 2, 0:HD],
                            vtp[:, 0:256].rearrange(
                                "p (c h d) -> p h c d", c=2, h=2))
                    items.append(vtrans)
                return items

            def pull(filler, n):
                for _ in range(n):
                    if not filler:
                        return
                    filler.popleft()()

            def make_ctx(rep, i):
                b, tqb = divmod(i, NB)
                tq0 = S * b + 512 * tqb
                q_aps = [qkt[64 * h:64 * h + 64, 0, tq0:tq0 + 512]
                         for h in range(2)]
                avs = [avp.tile([96, 512], f32, tag="av",
                                name=f"av_{rep}_{b}_{tqb}_{h}")
                       for h in range(2)]
                return dict(b=b, tqb=tqb, tq0=tq0, q_aps=q_aps, avs=avs)

            def emit_scores(rep, ctx, c):
                b, tqb = ctx["b"], ctx["tqb"]
                sc = scp.tile([128, 1024], f32, tag="sc",
                              name=f"sc_{rep}_{b}_{tqb}_{c}")
                for h in range(2):
                    k_ap = qkt[64 * h:64 * h + 64, 1,
                               S * b + 128 * c:S * b + 128 * c + 128]
                    nc.tensor.matmul(
                        sc[:, 512 * h:512 * h + 512], k_ap, ctx["q_aps"][h],
                        start=True, stop=(not use_mask))
                if use_mask:
                    mt = mkp.tile([128, 512], f32r, tag="mk",
                                  name=f"mk_{rep}_{b}_{tqb}_{c}")
                    nc.sync.dma_start(
                        mt, maskT[b, 128 * c:128 * c + 128,
                                  512 * tqb:512 * tqb + 512])
                    for h in range(2):
                        nc.tensor.matmul(
                            sc[:, 512 * h:512 * h + 512], id_sb, mt,
                            start=False, stop=True)
                return sc

            def queue_block_epilogue(rep, ctx, filler, last=False):
                """Normalize + output projection + out DMA for a finished
                block, all as filler items pulled during the next block.
                For the LAST block (nothing left to overlap) half the
                PSUM->SBUF casts run on the now-idle ScalarE and the out
                DMA is split so transfers overlap the remaining casts."""
                b, tqb, tq0, avs = ctx["b"], ctx["tqb"], ctx["tq0"], ctx["avs"]
                blk = b * NB + tqb

                if last:
                    # keep the PE clock warm through the ~5us DVE dance
                    # (a >3.4us PE idle drops the HAM gate to 1.2 GHz and
                    # the out-projection then runs at half clock)
                    def tail_warm():
                        wps = mmp.tile([128, 512], f32, tag="mm",
                                       name=f"tailwarm_{rep}")
                        for i in range(10):
                            nc.tensor.matmul(wps, warm_sb[:, 0:128], warm_sb,
                                             start=(i == 0), stop=(i == 9))
                        nc.vector.tensor_copy(warm_sb[0:1, 1:2],
                                              wps[0:1, 0:1])
                    filler.append(tail_warm)

                for h in range(2):
                    def norm(h=h):
                        # av rows 64..95 all hold l (32 ones columns in
                        # vext); 32x32 DVE block transposes make l
                        # partition-parallel because the DVE reciprocal
                        # costs ~6.5 cycles per FREE element (measured
                        # 3.3us for [1,512] -- the transposes are load-
                        # bearing), then a K=1 f32r PE matmul broadcasts
                        # 1/l across partitions for the DVE normalize.
                        # av is copied out of PSUM first so the (doubly-
                        # scarce) av buffer frees immediately: the NEXT
                        # block's AV accumulation reuses this buffer, and
                        # holding it head-blocks the in-order PE queue.
                        av = avs[h]
                        if not last:
                            av_sb = lrp.tile([64, 512], f32, tag="avs",
                                             name=f"avs_{rep}_{b}_{h}_{tqb}")
                            nc.vector.tensor_copy(av_sb, av[0:64, :])
                        lt = lrp.tile([96, 512], f32, tag="lt",
                                      name=f"lt_{rep}_{b}_{h}_{tqb}")
                        nc.vector.transpose(lt[64:96, :], av[64:96, :])
                        lt3 = lt[64:96, :].rearrange(
                            "p (a b) -> p a b", b=32)[:, :, 0:1]
                        nc.vector.reciprocal(lt3, lt3)
                        rlrowf = lrp.tile([96, 512], f32, tag="rlrowf",
                                          name=f"rlrowf_{rep}_{b}_{h}_{tqb}")
                        nc.vector.transpose(rlrowf[64:96, :], lt[64:96, :])
                        # 1/l broadcast in bf16 (f32r matmuls measured
                        # ~700ns vs ~320ns bf16; 1/l at bf16 adds ~0.2%
                        # relative -- rel err stays well under 2e-2).  All
                        # on DVE: GpSimd is ~3x slower on these small ops
                        # (measured 1861ns for a [1,512] cast) and this
                        # chain gates the next block's outproj filler.
                        rlrow = lrp.tile([65, 512], bf16, tag="rlrow",
                                         name=f"rlrow_{rep}_{b}_{h}_{tqb}")
                        nc.vector.tensor_copy(rlrow[64:65, :],
                                              rlrowf[64:65, :])
                        bcp = mmp.tile([128, 512], f32, tag="mm",
                                       name=f"bcp_{rep}_{b}_{tqb}_{h}")
                        nc.tensor.matmul(
                            bcp[0:64, :], ones_sb[64:65, :],
                            rlrow[64:65, :], start=True, stop=True)
                        if last:
                            # nothing left to overlap: stage the broadcast
                            # via the idle ScalarE and read av straight
                            # from PSUM (one-PSUM-operand rule), skipping
                            # the av_sb copy on the serial tail chain.
                            bcs = lrp.tile([64, 512], f32, tag="avs",
                                           name=f"bcs_{rep}_{b}_{h}_{tqb}")
                            nc.scalar.copy(bcs, bcp[0:64, :])
                            nc.vector.tensor_tensor(
                                valsT[64 * h:64 * h + 64, tq0:tq0 + 512],
                                bcs, av[0:64, :], MULT)
                        else:
                            nc.vector.tensor_tensor(
                                valsT[64 * h:64 * h + 64, tq0:tq0 + 512],
                                av_sb, bcp[0:64, :], MULT)
                    filler.append(norm)

                ot = otp.tile([128, 4, 2, 512], bf16, tag="ot",
                              name=f"ot_{rep}_{b}_{tqb}")
                late = []
                for u in range(4):
                    for nb2 in range(2):
                        def op_item(u=u, nb2=nb2):
                            t0 = tq0 + 128 * u
                            op = mmp.tile([128, 512], f32, tag="mm",
                                          name=f"op_{rep}_{b}_{tqb}_{nb2}_{u}")
                            nc.tensor.matmul(
                                op, valsT[:, t0:t0 + 128],
                                wo_sb[:, 512 * nb2:512 * nb2 + 512],
                                start=True, stop=True)
                            if last and nb2 == 1:
                                nc.scalar.copy(ot[:, u, nb2, :], op)
                            else:
                                nc.vector.tensor_copy(ot[:, u, nb2, :], op)
                        late.append(op_item)
                    if last:
                        # fire each quarter's DMA as soon as its casts are
                        # done: the final 1MB at ~137GB/s is otherwise a
                        # ~7us serial tail.
                        def quarter_dma(u=u):
                            if u == 3:
                                for nb2 in range(2):
                                    nc.sync.dma_start(
                                        out[blk, :, u:u + 1, nb2:nb2 + 1, :],
                                        ot[:, u:u + 1, nb2:nb2 + 1, :])
                            else:
                                nc.sync.dma_start(out[blk, :, u:u + 1, :, :],
                                                  ot[:, u:u + 1, :, :])
                        late.append(quarter_dma)

                if not last:
                    def out_dma():
                        nc.sync.dma_start(out[blk, :, :, :, :], ot)
                    late.append(out_dma)
                return late

            for rep in range(reps):
                filler = deque()
                # Only tb0 of Phase A is emitted directly: attention(b0,
                # tqb0) needs just the first k/v chunks, so tb1-3 stream in
                # as filler during its chunk loop (chunk 4c needs tb c,
                # pulled 2 items/chunk -> arrives just in time).  Batch 1's
                # blocks become filler for the later b0 attention blocks.
                for item in phase_a_block(rep, 0, 0, first=True):
                    item()
                for tb in range(1, 4):
                    # fire tb1-3's x DMAs now (right behind the weights in
                    # the queue; the relayout makes each a ~3us contiguous
                    # transfer) -- chunk 4c of b0 attention needs tb c.
                    items = phase_a_block(rep, 0, tb)
                    items.pop(0)()
                    filler.extend(items)

                # Flat (block, chunk) sequence with scores emitted DEPTH=3
                # chunks ahead of AV (across block boundaries): the exp
                # stream only depends on scores, and when heavy Phase-A
                # filler makes the in-order PE queue lag, a 1-deep score
                # pipeline lags with it and stalls ScalarE.  Depth 3 plus
                # the 8-deep pt pool decouples exp from PE lag.
                DEPTH = 3
                seq = [(i, c) for i in range(NBLK) for c in range(NCH)]
                ctxs = {}

                def get_ctx(i):
                    if i not in ctxs:
                        ctxs[i] = make_ctx(rep, i)
                    return ctxs[i]

                sc_q = deque()
                pending_epi = []
                for j in range(DEPTH):
                    sc_q.append(emit_scores(rep, get_ctx(seq[j][0]),
                                            seq[j][1]))
                for idx, (i, c) in enumerate(seq):
                    if idx + DEPTH < len(seq):
                        i2, c2 = seq[idx + DEPTH]
                        sc_q.append(emit_scores(rep, get_ctx(i2), c2))
                    if c == 5 and pending_epi:
                        # the previous block's out-projection waits on its
                        # normalize (a ~5us DVE chain); pulled any earlier
                        # it head-blocks this block's scores in the
                        # in-order PE queue and stalls the exp stream.
                        filler.extend(pending_epi)
                        pending_epi = []
                    pull(filler, 2)
                    ctx = get_ctx(i)
                    pt = ptp.tile([128, 1024], bf16, tag="pt",
                                  name=f"pt_{rep}_{i}_{c}")
                    nc.scalar.activation(pt, sc_q.popleft(), EXP)
                    for h in range(2):
                        nc.tensor.matmul(
                            ctx["avs"][h], vext[:, ctx["b"], h, c, :],
                            pt[:, 512 * h:512 * h + 512],
                            start=(c == 0), stop=(c == NCH - 1))
                    if c == NCH - 1:
                        pending_epi = queue_block_epilogue(
                            rep, ctx, filler, last=(i == NBLK - 1))
                        if ctx["b"] == 0:
                            filler.extend(
                                phase_a_block(rep, 1, 4 + ctx["tqb"]))
                filler.extend(pending_epi)
                while filler:
                    filler.popleft()()
    nc.compile()
    return nc


def make_in_maps(mha_x, self_mask, w_qkv, b_qkv, w_out, b_out, use_mask):
    """Host-side sharding / layout prep. Returns (in_maps, host_bias)."""
    import ml_dtypes
    bf = np.dtype(ml_dtypes.bfloat16)
    x = np.asarray(mha_x, np.float32).reshape(T, D)
    # x^T pre-arranged [tb, p, c, j] so each 512-token block is one
    # contiguous-per-partition DMA (8KB rows; the naive (c p) j gather
    # ran at ~73 GB/s, ~8us for 512KB)
    xTb_np = np.ascontiguousarray(
        x.T.reshape(8, 128, 8, 512).transpose(2, 1, 0, 3).astype(bf))
    scale = 1.0 / np.sqrt(np.float32(HD))               # 1/8
    wqkv = np.asarray(w_qkv, np.float32)
    bqkv = np.asarray(b_qkv, np.float32)
    wout = np.asarray(w_out, np.float32)
    bout = np.asarray(b_out, np.float32)

    # reference packs w_qkv rows as [H, (q,k,v), HD]: head h's q rows are
    # wqkv[192h:192h+64], k rows +64, v rows +128.
    wq_rows = lambda h: wqkv[192 * h:192 * h + 64, :]
    wk_rows = lambda h: wqkv[192 * h + 64:192 * h + 128, :]
    wv_rows = lambda h: wqkv[192 * h + 128:192 * h + 192, :]
    bq_of = lambda h: bqkv[192 * h:192 * h + 64]
    bk_of = lambda h: bqkv[192 * h + 64:192 * h + 128]
    bv_of = lambda h: bqkv[192 * h + 128:192 * h + 192]

    in_maps = []
    for c in range(NCORES):
        h0, h1 = 2 * c, 2 * c + 1
        wq = np.concatenate([wq_rows(h0), wq_rows(h1)], 0) * scale
        wk = np.concatenate([wk_rows(h0), wk_rows(h1)], 0)
        wv = np.concatenate([wv_rows(h0), wv_rows(h1)], 0)
        m = {
            "xTb": xTb_np,
            "wqkT": np.ascontiguousarray(
                np.concatenate([wq, wk], 0).T.reshape(
                    8, 128, 256).transpose(1, 0, 2).astype(bf)),
            "bqk": np.ascontiguousarray(
                np.stack([np.concatenate([bq_of(h0), bq_of(h1)]) * scale,
                          np.concatenate([bk_of(h0), bk_of(h1)])], 1)),
            "wvT": np.ascontiguousarray(
                wv.T.reshape(8, 128, 128).transpose(1, 0, 2).astype(bf)),
            "woT": np.ascontiguousarray(
                wout[:, 128 * c:128 * c + 128].T.astype(bf)),
            "identb": np.eye(128, dtype=np.float32).astype(bf),
        }
        if use_mask:
            m["maskT"] = np.ascontiguousarray(
                np.asarray(self_mask, np.float32).transpose(0, 2, 1))
            m["ident"] = np.eye(128, dtype=np.float32)
        in_maps.append(m)

    b_v_full = np.concatenate([bv_of(h) for h in range(H)])
    host_bias = b_v_full @ wout.T + bout                # [D], exact
    return in_maps, host_bias


def kernel(**inputs):
    from concourse.bass_utils import run_bass_kernel_spmd
    self_mask = np.asarray(inputs["self_mask"], np.float32)
    use_mask = bool(np.any(self_mask))
    key = ("nc", use_mask)
    if key not in _CACHE:
        _CACHE[key] = build_nc(use_mask)
    nc = _CACHE[key]
    in_maps, host_bias = make_in_maps(
        inputs["mha_x"], self_mask, inputs["w_qkv"], inputs["b_qkv"],
        inputs["w_out"], inputs["b_out"], use_mask)
    res = run_bass_kernel_spmd(nc, in_maps, core_ids=list(range(NCORES)))
    acc = np.zeros((NBLK, 512, D), np.float32)
    for c in range(NCORES):
        # out is [blk, p, u, nb2, j]; token = 128u+p, feature = 512nb2+j
        arr = res.results[c]["out"].astype(np.float32)
        acc += arr.transpose(0, 2, 1, 3, 4).reshape(NBLK, 512, D)
    acc += host_bias[None, None, :]
    return acc.reshape(B, S, D)
